# revision 24
# baseline (speedup 1.0000x reference)
"""Trainium2 Bass kernel for nn_CascadeDEDBackbone (ResNet-style encoder/decoder,
[2,128,256,256] f32, all convs 128->128ch).

Strategy (self-contained, hardcoded):
  - 8 cores = batch(2) x H-half(2) x W-half(2). Host flip-normalizes every
    tile so its owned 128x128 anchors at local (0,0) (weights flipped per
    core to compensate); each conv computes only the region later stages
    consume (validity shrinks 1px/conv toward the high side).
  - Stride-2 convs are not flip-equivariant (sampling phase), so they use a
    4x4 tap window (16 matmuls) with each core's flipped 3x3 weights placed
    at offset (i,j) by the host; 1x1 downsamples use a 2x2 window. This
    keeps one SPMD program for all 8 cores.
  - Input margin is 7px (vs 24 fully-redundant): two halo exchanges refill
    margins mid-net via three CONCURRENT 2-rank AllReduces (H-pair rows,
    W-pair cols, diagonal-pair corner; +local subtract, exact in f32):
    exchange A after e1b0 (margin 2 at 128-res), exchange B on f1
    (margin 7). Producers emit boundary chunks first and consumers emit
    all interior chunks before any margin strip (the PE stream executes
    in emission order), so the collectives hide under interior compute.
    Exchange DMAs ride gpsimd/SWDGE lanes to dodge the contended HWDGE
    ring-reuse waits (one-wait ISA limit).
  - On-core: a KxK conv = K^2 accumulated 128x128 matmuls over shifted APs
    (channels = partitions). PSUM accumulates f32; VectorE evacuates with
    fused relu/scale/residual-add; activations stay resident in SBUF.
  - Decoder evacuation alternates per chunk between DVE and an ACT->Pool
    chain so the deconv stages stay PE-bound; output is bf16 128x128/core.
"""

import os
import sys

import numpy as np
import ml_dtypes

for _p in ("/opt/trn_rl_repo", "/opt/trn_rl_repo/concourse"):
    if os.path.isdir(_p) and _p not in sys.path:
        sys.path.insert(0, _p)

BF16 = ml_dtypes.bfloat16
BN_S = float(1.0 / np.sqrt(1.0 + 1e-3))

# region geometry: owned 128, host margin 7 (incl 1-px ring -> 136 input)
M0 = 7
B0 = 136
REG_L0 = [134, 133, 132, 131]   # e0b0c1, e0b0c2, e0b1c1, e0b1c2(f0)
REG_L1 = [65, 64, 65, 64]       # e1b0c1, e1b0c2, e1b1c1, e1b1c2(f1)
REG_L2 = [35, 34, 33, 32]       # e2b0c1, e2b0c2, e2b1c1, e2b1c2(e2o)
MA = 2                          # exchange A margin (bo1, 128-res)
MB = 7                          # exchange B margin (f1p, 128-res)
DEC0_IN = 32
DEC1_IN = 64
OWNED = 128
RS2 = [0, 122]                  # host slice start per tile index
OWN = [0, 128]

OFF9 = [(dy, dx) for dy in range(3) for dx in range(3)]
OFF16 = [(dy, dx) for dy in range(4) for dx in range(4)]
OFF4 = [(k, l) for k in range(2) for l in range(2)]

GRP_H = [[0, 2], [1, 3], [4, 6], [5, 7]]
GRP_W = [[0, 1], [2, 3], [4, 5], [6, 7]]
GRP_D = [[0, 3], [1, 2], [4, 7], [5, 6]]

W3_NAMES = ['e0b0c1', 'e0b0c2', 'e0b1c1', 'e0b1c2',
            'e1b0c2', 'e1b1c1', 'e1b1c2',
            'e2b0c2', 'e2b1c1', 'e2b1c2']
W16_NAMES = ['e1b0c1', 'e2b0c1']
# BN-scale folding: weights consuming pre-scaled f0'/f1' buffers get their
# BN fold cancelled.
W_SCALE = {'e0b0c1': BN_S, 'e0b0c2': BN_S, 'e0b1c1': BN_S, 'e0b1c2': BN_S,
           'e1b0c1': 1.0, 'e1b0c2': BN_S, 'e1b0ds': 1.0,
           'e1b1c1': BN_S, 'e1b1c2': BN_S,
           'e2b0c1': 1.0, 'e2b0c2': BN_S, 'e2b0ds': 1.0,
           'e2b1c1': BN_S, 'e2b1c2': BN_S,
           'dec0w': BN_S * BN_S, 'dec1w': BN_S * BN_S}

_W_ORDER = ['e0b0c1', 'e0b0c2', 'e0b1c1', 'e0b1c2',
            'e1b0c1', 'e1b0c2', 'e1b0ds', 'e1b1c1', 'e1b1c2',
            'e2b0c1', 'e2b0c2', 'e2b0ds', 'e2b1c1', 'e2b1c2',
            'dec0w', 'dec1w']
_W_LENS = {**{n: 9 * 128 for n in W3_NAMES},
           **{n: 16 * 128 for n in W16_NAMES},
           'e1b0ds': 4 * 128, 'e2b0ds': 4 * 128,
           'dec0w': 4 * 128, 'dec1w': 4 * 128}
WPACK_OFFS = []
_off = 0
for _n in _W_ORDER:
    WPACK_OFFS.append((_n, _off, _W_LENS[_n]))
    _off += _W_LENS[_n]
WPACK_LEN = _off

_PROGRAM = None  # cached bass.Bass


def _build_program(n_repeat=None):
    import concourse.bass as bass
    import concourse.mybir as mybir
    import concourse.tile as tile
    from contextlib import ExitStack

    if n_repeat is None:
        n_repeat = int(os.environ.get("K_FULL_REPEAT", "1"))
    use_cc = os.environ.get("K_NO_CC", "0") != "1"

    bf = mybir.dt.bfloat16
    f32 = mybir.dt.float32
    ADD = mybir.AluOpType.add
    SUB = mybir.AluOpType.subtract
    MAX = mybir.AluOpType.max
    MULT = mybir.AluOpType.mult

    nc = bass.Bass()

    xt_d = nc.dram_tensor("xt", [128, B0, B0], bf, kind="ExternalInput")
    wpack_d = nc.dram_tensor("wpack", [128, WPACK_LEN], bf, kind="ExternalInput")
    out_d = nc.dram_tensor("out", [128, OWNED, OWNED], bf,
                           kind="ExternalOutput")

    with tile.TileContext(nc) as tc, ExitStack() as ctx:
        wp = ctx.enter_context(tc.tile_pool(name="wpool", bufs=1))
        wslab = wp.tile([128, WPACK_LEN], bf, tag="wpack", name="wslab")
        w3 = {}
        for n, off, ln in WPACK_OFFS:
            view = wslab[:, off: off + ln]
            w3[n] = view.rearrange("c (n m) -> c n m", n=ln // 128)

        def dma_w(n):
            _, off, ln = next(t for t in WPACK_OFFS if t[0] == n)
            nc.sync.dma_start(wslab[:, off: off + ln],
                              wpack_d[:, off: off + ln])

        pers = ctx.enter_context(tc.tile_pool(name="pers", bufs=1))

        psp = ctx.enter_context(
            tc.tile_pool(name="psp", bufs=4, space=bass.MemorySpace.PSUM))
        tmpp = ctx.enter_context(tc.tile_pool(name="tmpp", bufs=8))
        dramp = ctx.enter_context(
            tc.tile_pool(name="dramp", bufs=1, space="DRAM"))
        xch = ctx.enter_context(tc.tile_pool(name="xch", bufs=1))

        # DMA order: first conv's weights, then the input in row bands (so
        # the first conv chunks start as soon as their rows land), then the
        # remaining weights. After each band, a tiny DVE read (absorber)
        # publishes the band's DMA completion into DVE's vector clock.
        _, _w0off, _ = next(t for t in WPACK_OFFS if t[0] == 'e0b0c1')
        nc.sync.dma_start(wslab[:, _w0off: _w0off + 128],
                          wpack_d[:, _w0off: _w0off + 128])
        nc.sync.dma_start(wslab[:, _w0off + 128: _w0off + 9 * 128],
                          wpack_d[:, _w0off + 128: _w0off + 9 * 128])

        def dma_input(xt):
            bands = [0, 3, 6] + list(range(26, B0, 20)) + [B0]
            for a, b_hi in zip(bands[:-1], bands[1:]):
                nc.sync.dma_start(xt[:, a:b_hi, :], xt_d[:, a:b_hi, :])
                scratch = tmpp.tile([128, 1, 1], bf, tag="scratch",
                                    name="scratch")
                nc.vector.tensor_copy(scratch[:], xt[:, b_hi - 1:b_hi, 0:1])

        xt = pers.tile([128, B0, B0], bf, tag="pers", name="xt_s")
        dma_input(xt)
        _w_emit = ['dec1w', 'dec0w', 'e2b0ds', 'e2b1c2', 'e2b1c1', 'e2b0c2',
                   'e2b0c1', 'e1b0ds', 'e1b1c2', 'e1b1c1', 'e1b0c2', 'e1b0c1',
                   'e0b1c2', 'e0b1c1', 'e0b0c2']
        assert set(_w_emit) == {n for n, _, _ in WPACK_OFFS} - {'e0b0c1'}
        for n in _w_emit:
            dma_w(n)
        del dma_w

        def ring_zero(t, H):
            # only the low-side ring (image-edge zero padding) is read
            nc.vector.memset(t[:, 0, :], 0.0)
            nc.vector.memset(t[:, 1:, 0], 0.0)

        def conv3x3(src, dst, w, stride, rchunk, y_off=0, hr=None, x0=0,
                    cw=None, reg=None, scale=1.0, resid=None, extra_mm=None,
                    taps=OFF9):
            """Compute out rows y_off..y_off+hr-1, cols x0..x0+cw-1
            (0-based within interior; dst row = 1+y). reg: full region (for
            defaults). resid: callable (y0, rr, x0, cw) -> identity AP."""
            hr = reg - y_off if hr is None else hr
            cw = reg - x0 if cw is None else cw
            if resid is not None:
                ab = tmpp.tile([128, 1, cw], bf, tag="absorb", name="absorb")
                nc.vector.tensor_copy(ab[:], resid(y_off + hr - 1, 1, x0, cw))
            n_ext = 0 if extra_mm is None else 4
            last = len(taps) + n_ext - 1
            for y0 in range(y_off, y_off + hr, rchunk):
                rr = min(rchunk, y_off + hr - y0)
                ps = psp.tile([128, rr, cw], f32, tag="ps", name="ps")
                for o, (dy, dx) in enumerate(taps):
                    r_lo = y0 * stride + dy
                    c_lo = x0 * stride + dx
                    rhs = src[:, r_lo: r_lo + (rr - 1) * stride + 1: stride,
                              c_lo: c_lo + (cw - 1) * stride + 1: stride]
                    nc.tensor.matmul(ps[:], w[:, o, :], rhs,
                                     start=(o == 0), stop=(o == last))
                if extra_mm is not None:
                    w4, src2 = extra_mm
                    for q, (dy, dx) in enumerate(OFF4):
                        rhs = src2[:, 1 + 2 * y0 + dy:
                                   1 + 2 * y0 + dy + (rr - 1) * 2 + 1: 2,
                                   1 + 2 * x0 + dx:
                                   1 + 2 * x0 + dx + (cw - 1) * 2 + 1: 2]
                        nc.tensor.matmul(ps[:], w4[:, q, :], rhs,
                                         start=False,
                                         stop=(len(taps) + q == last))
                dst_ap = dst[:, 1 + y0: 1 + y0 + rr, 1 + x0: 1 + x0 + cw]
                if resid is None:
                    nc.vector.tensor_scalar(dst_ap, ps[:], 0.0, scale, MAX,
                                            MULT)
                else:
                    tmp = tmpp.tile([128, rr, cw], bf, tag="tmpr", name="tmpr")
                    nc.vector.tensor_tensor(tmp[:], ps[:],
                                            resid(y0, rr, x0, cw), op=ADD)
                    nc.vector.tensor_scalar(dst_ap, tmp[:], 0.0, scale, MAX,
                                            MULT)

        def interior(buf):
            return lambda y0, rr, x0, cw: buf[:, 1 + y0: 1 + y0 + rr,
                                              1 + x0: 1 + x0 + cw]

        def _ar_chain(ox, shp, grp, sfx, tag):
            ci = dramp.tile(shp, f32, tag=f"ci{sfx}{tag}")
            co = dramp.tile(shp, f32, tag=f"co{sfx}{tag}")
            nc.gpsimd.dma_start(ci[:], ox[:])
            if use_cc:
                nc.gpsimd.collective_compute(
                    "AllReduce", ADD, replica_groups=grp,
                    ins=[ci.opt()], outs=[co.opt()])
            else:
                nc.sync.dma_start(co[:], ci[:])
            return co

        def exchange_hd(buf, n, m, tag):
            """Early phase: launch the H-row and diagonal-corner AllReduces.
            Their payloads need only the producer's BOUNDARY chunk (emitted
            first), so issuing here - before the interior conv evacs enter
            the DVE stream - gives the collectives the whole interior
            compute as flight time. SPMD-safe via AllReduce(add) of f32 +
            local subtract (exact)."""
            oxh = xch.tile([128, m, n], f32, tag=f"oxh{tag}", name="oxh")
            for r in range(m):
                nc.vector.tensor_copy(oxh[:, r, :], buf[:, n - r, 1:1 + n])
            oxd = xch.tile([128, m, m], f32, tag=f"oxd{tag}", name="oxd")
            for r in range(m):
                nc.vector.tensor_copy(oxd[:, r, :],
                                      buf[:, n - r, n - m + 1: n + 1])
            coh = _ar_chain(oxh, [128, m, n], GRP_H, "h", tag)
            cod = _ar_chain(oxd, [128, m, m], GRP_D, "d", tag)
            return oxh, coh, oxd, cod

        def exchange_w(buf, n, m, tag, hd):
            """Late phase: W-col AllReduce (needs every row chunk of buf),
            then apply all three margins. W outbox stored TRANSPOSED [m, n]
            so the bounce DMAs burst well (28B inner runs DMA ~5x slower).
            Corner: ox/in col index c holds neighbor col n-c; written into
            margin cols reversed so receiver col n+1+c' = diag col n-c'."""
            oxh, coh, oxd, cod = hd
            oxw = xch.tile([128, m, n], f32, tag=f"oxw{tag}", name="oxw")
            for c in range(m):
                nc.vector.tensor_copy(oxw[:, c, :], buf[:, 1: 1 + n, n - c])
            cow = _ar_chain(oxw, [128, m, n], GRP_W, "w", tag)
            inh = xch.tile([128, m, n], f32, tag=f"inh{tag}", name="inh")
            nc.gpsimd.dma_start(inh[:], coh[:])
            nc.vector.tensor_tensor(buf[:, n + 1: n + 1 + m, 1: 1 + n],
                                    inh[:], oxh[:], op=SUB)
            ind = xch.tile([128, m, m], f32, tag=f"ind{tag}", name="ind")
            nc.gpsimd.dma_start(ind[:], cod[:])
            for c in range(m):
                nc.vector.tensor_tensor(
                    buf[:, n + 1: n + 1 + m, n + 1 + c],
                    ind[:, :, m - 1 - c], oxd[:, :, m - 1 - c], op=SUB)
            inw = xch.tile([128, m, n], f32, tag=f"inw{tag}", name="inw")
            nc.gpsimd.dma_start(inw[:], cow[:])
            for c in range(m):
                nc.vector.tensor_tensor(buf[:, 1: 1 + n, n + 1 + c],
                                        inw[:, c, :], oxw[:, c, :], op=SUB)

        def body(rep, xt):
            rtag = f"r{rep}" if rep else ""
            # ------------- level 0 (256-res, regions 134..131) -----------
            with tc.tile_pool(name=f"work{rtag}", bufs=2) as workp:
                c1 = workp.tile([128, B0, B0], bf, tag="work", name="b0c1")
                ring_zero(c1, B0 - 2)
                for _rep in range(int(os.environ.get("K_CAL_REPEAT", "1"))):
                    conv3x3(xt, c1, w3['e0b0c1'], 1, 3, reg=REG_L0[0])
                b0 = workp.tile([128, B0, B0], bf, tag="work", name="b0out")
                ring_zero(b0, B0 - 2)
                conv3x3(c1, b0, w3['e0b0c2'], 1, 3, reg=REG_L0[1],
                        resid=interior(xt))
                c1 = workp.tile([128, B0, B0], bf, tag="work", name="b1c1")
                ring_zero(c1, B0 - 2)
                conv3x3(b0, c1, w3['e0b1c1'], 1, 3, reg=REG_L0[2])
                f0p = pers.tile([128, B0, B0], bf, tag="pers",
                                name="f0p")
                ring_zero(f0p, B0 - 2)
                conv3x3(c1, f0p, w3['e0b1c2'], 1, 3, reg=REG_L0[3],
                        scale=BN_S, resid=interior(b0))

            # ------------- levels 1+2 / decoder pools -------------
            with tc.tile_pool(name=f"acts2{rtag}", bufs=1) as acts2, \
                 tc.tile_pool(name=f"lv{rtag}", bufs=1) as lv, \
                 tc.tile_pool(name=f"ostp{rtag}", bufs=6) as ostp, \
                 tc.tile_pool(name=f"tmpa{rtag}", bufs=4) as tmpa, \
                 tc.tile_pool(name=f"dpsp{rtag}", bufs=4,
                              space=bass.MemorySpace.PSUM) as dpsp:

                # ---- level 1 block 0 (stride-2, 16-tap window) ----
                c1 = lv.tile([128, 67, 67], bf, tag="c1", name="c1")
                ring_zero(c1, 65)
                conv3x3(f0p, c1, w3['e1b0c1'], 2, 6, reg=REG_L1[0],
                        taps=OFF16)
                bo1 = lv.tile([128, 67, 67], bf, tag="bo1", name="bo1")
                ring_zero(bo1, 65)
                # boundary rows first so exchange A starts early
                conv3x3(c1, bo1, w3['e1b0c2'], 1, 6, y_off=60, hr=4,
                        reg=REG_L1[1], extra_mm=(w3['e1b0ds'], f0p))
                hdA = exchange_hd(bo1, 64, MA, f"A{rtag}")
                conv3x3(c1, bo1, w3['e1b0c2'], 1, 6, y_off=0, hr=60,
                        reg=REG_L1[1], extra_mm=(w3['e1b0ds'], f0p))
                exchange_w(bo1, 64, MA, f"A{rtag}", hdA)

                # ---- level 1 block 1 (interior + margin strips) ----
                c1b = lv.tile([128, 67, 67], bf, tag="b1c1", name="b1c1")
                ring_zero(c1b, 65)
                conv3x3(bo1, c1b, w3['e1b1c1'], 1, 6, y_off=0, hr=63,
                        x0=0, cw=63, reg=REG_L1[2])
                conv3x3(bo1, c1b, w3['e1b1c1'], 1, 63, y_off=0, hr=63,
                        x0=63, cw=2, reg=REG_L1[2])
                conv3x3(bo1, c1b, w3['e1b1c1'], 1, 6, y_off=63, hr=2,
                        x0=0, cw=65, reg=REG_L1[2])
                f1p = acts2.tile([128, 72, 72], bf, tag="f1p", name="f1p")
                ring_zero(f1p, 70)
                conv3x3(c1b, f1p, w3['e1b1c2'], 1, 6, y_off=54, hr=10,
                        reg=REG_L1[3], scale=BN_S, resid=interior(bo1))
                hdB = exchange_hd(f1p, 64, MB, f"B{rtag}")
                conv3x3(c1b, f1p, w3['e1b1c2'], 1, 6, y_off=0, hr=54,
                        reg=REG_L1[3], scale=BN_S, resid=interior(bo1))
                exchange_w(f1p, 64, MB, f"B{rtag}", hdB)

                # ---- level 2 ----
                # Emit ALL interiors first, then all margin strips: the PE
                # stream runs in emission order, so strips (which wait on
                # exchange B) must sit behind every interior chunk to avoid
                # head-of-line blocking while the collectives fly.
                c2 = lv.tile([128, 37, 37], bf, tag="c1", name="c2")
                ring_zero(c2, 35)
                bo2 = lv.tile([128, 36, 36], bf, tag="bo1", name="bo2")
                ring_zero(bo2, 34)
                c2b = lv.tile([128, 35, 35], bf, tag="b1c1", name="b2c1")
                ring_zero(c2b, 33)
                e2o = acts2.tile([128, 34, 34], bf, tag="e2o", name="e2o")
                ring_zero(e2o, 32)
                conv3x3(f1p, c2, w3['e2b0c1'], 2, 13, y_off=0, hr=31,
                        x0=0, cw=31, reg=REG_L2[0], taps=OFF16)
                conv3x3(c2, bo2, w3['e2b0c2'], 1, 13, y_off=0, hr=30,
                        x0=0, cw=30, reg=REG_L2[1],
                        extra_mm=(w3['e2b0ds'], f1p))
                conv3x3(bo2, c2b, w3['e2b1c1'], 1, 13, y_off=0, hr=29,
                        x0=0, cw=29, reg=REG_L2[2])
                conv3x3(c2b, e2o, w3['e2b1c2'], 1, 13, y_off=0, hr=28,
                        x0=0, cw=28, reg=REG_L2[3], resid=interior(bo2))
                # strips (exchange-B-dependent cascade)
                conv3x3(f1p, c2, w3['e2b0c1'], 2, 31, y_off=0, hr=31,
                        x0=31, cw=4, reg=REG_L2[0], taps=OFF16)
                conv3x3(f1p, c2, w3['e2b0c1'], 2, 13, y_off=31, hr=4,
                        x0=0, cw=35, reg=REG_L2[0], taps=OFF16)
                conv3x3(c2, bo2, w3['e2b0c2'], 1, 30, y_off=0, hr=30,
                        x0=30, cw=4, reg=REG_L2[1],
                        extra_mm=(w3['e2b0ds'], f1p))
                conv3x3(c2, bo2, w3['e2b0c2'], 1, 13, y_off=30, hr=4,
                        x0=0, cw=34, reg=REG_L2[1],
                        extra_mm=(w3['e2b0ds'], f1p))
                conv3x3(bo2, c2b, w3['e2b1c1'], 1, 29, y_off=0, hr=29,
                        x0=29, cw=4, reg=REG_L2[2])
                conv3x3(bo2, c2b, w3['e2b1c1'], 1, 13, y_off=29, hr=4,
                        x0=0, cw=33, reg=REG_L2[2])
                conv3x3(c2b, e2o, w3['e2b1c2'], 1, 28, y_off=0, hr=28,
                        x0=28, cw=4, reg=REG_L2[3], resid=interior(bo2))
                conv3x3(c2b, e2o, w3['e2b1c2'], 1, 13, y_off=28, hr=4,
                        x0=0, cw=32, reg=REG_L2[3], resid=interior(bo2))

                # ---------------- decoder ----------------
                RELU = mybir.ActivationFunctionType.Relu
                COPY = mybir.ActivationFunctionType.Copy

                pool_px = [None]

                def act_absorb():
                    if pool_px[0] is not None:
                        ab = tmpa.tile([128, 1], bf, tag="actab", name="actab")
                        nc.scalar.activation(ab[:], pool_px[0], COPY)

                # stage 0: y1 = BN_S*relu(deconv(e2o, dec0w*s)) + f1p
                D0, D1 = DEC0_IN, DEC1_IN
                y1 = acts2.tile([128, D1, D1], bf, tag="y1", name="y1")
                use_chain = os.environ.get("K_DEC_CHAIN", "1") == "1"
                for ci, y0 in enumerate(range(0, D0, 9)):
                    chain = (ci % 2 == 1) and use_chain
                    if chain:
                        act_absorb()
                    for p, (k, l) in enumerate(OFF4):
                        rr = min(9, D0 - y0)
                        pool = psp if p % 2 == 0 else dpsp
                        tag = "ps" if p % 2 == 0 else "dps"
                        ps = pool.tile([128, rr, D0], f32, tag=tag, name="ps")
                        nc.tensor.matmul(ps[:], w3['dec0w'][:, p, :],
                                         e2o[:, 1 + y0: 1 + y0 + rr, 1: 1 + D0])
                        y1_ap = y1[:, 2 * y0 + k: 2 * (y0 + rr - 1) + k + 1: 2,
                                   l: 2 * (D0 - 1) + l + 1: 2]
                        f1_ap = f1p[:, 1 + 2 * y0 + k:
                                    1 + 2 * (y0 + rr - 1) + k + 1: 2,
                                    1 + l: 1 + 2 * (D0 - 1) + l + 1: 2]
                        # s^2 folded into dec0w: relu(s^2 psum) = s relu(s psum)
                        if not chain:
                            nc.vector.scalar_tensor_tensor(
                                y1_ap, ps[:], 0.0, f1_ap, op0=MAX, op1=ADD)
                        else:
                            tmp = tmpa.tile([128, rr, D0], bf, tag="t0",
                                            name="t0", bufs=8)
                            nc.scalar.activation(tmp[:], ps[:], RELU)
                            if p == 0:
                                pab = tmpa.tile([128, 1], bf,
                                                tag=f"pab0_{ci}{rtag}",
                                                name="pab", bufs=1)
                                nc.gpsimd.tensor_copy(pab[:], tmp[:, 0, 0:1])
                            nc.gpsimd.tensor_tensor(y1_ap, tmp[:], f1_ap,
                                                    op=ADD)
                            pool_px[0] = y1[:, 2 * y0 + k: 2 * y0 + k + 1,
                                            l: l + 1]

                # stage 1: out = BN_S*relu(deconv(y1, dec1w*s)) + f0p
                ost_tiles = []
                SUB6 = 6
                nch = (D1 + SUB6 - 1) // SUB6

                def ensure_ost(i):
                    while len(ost_tiles) <= min(i, nch - 1):
                        j = len(ost_tiles)
                        rrj = min(SUB6, D1 - j * SUB6)
                        t = ostp.tile([128, 2 * rrj, OWNED], bf, tag="ost",
                                      name="ost")
                        ost_tiles.append(t)
                        if j % 2 == 0:
                            nc.vector.memset(t[:, 0:1, 0:1], 0.0)
                        else:
                            nc.gpsimd.memset(t[:, 0:1, 0:1], 0.0)

                for ci, y0 in enumerate(range(0, D1, SUB6)):
                    ensure_ost(ci + 4)
                    chain = (ci % 2 == 1) and use_chain
                    rr = min(SUB6, D1 - y0)
                    ost = ost_tiles[ci]
                    if chain:
                        act_absorb()
                    for p, (k, l) in enumerate(OFF4):
                        ps = dpsp.tile([128, rr * D1], f32, tag="dps",
                                       name="dps")
                        nc.tensor.matmul(ps[:], w3['dec1w'][:, p, :],
                                         y1[:, y0: y0 + rr, :])
                        f0_ap = f0p[:, 1 + 2 * y0 + k:
                                    1 + 2 * (y0 + rr - 1) + k + 1: 2,
                                    1 + l: 1 + 2 * (D1 - 1) + l + 1: 2]
                        ost_ap = ost[:, k: 2 * (rr - 1) + k + 1: 2,
                                     l: 2 * (D1 - 1) + l + 1: 2]
                        ps_2d = ps[:].rearrange("c (r w) -> c r w", r=rr)
                        # s^2 folded into dec1w
                        if not chain:
                            nc.vector.scalar_tensor_tensor(
                                ost_ap, ps_2d, 0.0, f0_ap, op0=MAX, op1=ADD)
                        else:
                            tmp = tmpa.tile([128, rr * D1], bf, tag="t1",
                                            name="t1", bufs=8)
                            nc.scalar.activation(tmp[:], ps[:], RELU)
                            if p == 0:
                                pab = tmpa.tile([128, 1], bf,
                                                tag=f"pab1_{ci}{rtag}",
                                                name="pab", bufs=1)
                                nc.gpsimd.tensor_copy(pab[:], tmp[:, 0:1])
                            tmp_2d = tmp[:].rearrange("c (r w) -> c r w", r=rr)
                            nc.gpsimd.tensor_tensor(ost_ap, tmp_2d, f0_ap,
                                                    op=ADD)
                            pool_px[0] = ost[:, k: k + 1, l: l + 1]
                    nc.sync.dma_start(out_d[:, 2 * y0: 2 * y0 + 2 * rr, :],
                                      ost[:])

                # Tail absorbers (see baseline notes): engine-matched writes
                # to the final ost slots carry the WAR waits on their output
                # DMAs, then a DVE read of the last Pool-written slot pulls
                # Pool's clock into DVE's.
                last_chain_t = None
                n_tail = len(ost_tiles) - 6
                for ti in range(max(0, n_tail), len(ost_tiles)):
                    t = ost_tiles[ti]
                    if ti % 2 == 1:
                        nc.gpsimd.memset(t[:, 0:1, 0:1], 0.0)
                        last_chain_t = t
                    else:
                        nc.vector.memset(t[:, 0:1, 0:1], 0.0)
                if last_chain_t is not None:
                    scratch = tmpp.tile([128, 1, 1], bf, tag="scratch",
                                        name="scratch")
                    nc.vector.tensor_copy(scratch[:],
                                          last_chain_t[:, 0:1, 0:1])
                scratch = tmpp.tile([128, 1, B0], bf, tag="scratch_row",
                                    name="scratch_row")
                nc.vector.tensor_copy(scratch[:], f0p[:, 0:1, :])

        body(0, xt)
        for rep in range(1, n_repeat):
            xt = pers.tile([128, B0, B0], bf, tag="pers",
                           name=f"xt_s{rep}")
            dma_input(xt)
            body(rep, xt)

    _legalize_waits(nc, mybir)
    nc.finalize()
    return nc


def _legalize_waits(nc, mybir):
    """Drop semaphore waits provably implied by other synchronization.

    Compute-engine and DMA ISA structs fit only one sync wait, but Tile's
    sem assignment is per-proc minimal, not transitively minimal. We replay
    the schedule with a vector-clock and drop implied waits. DMACopy
    dispatch is asynchronous (DGE evaluates its waits, not the issuing
    engine), so DMAs contribute nothing to engine knowledge and get no
    dispatch-order credit.
    """
    QDEPTH_PE, QDEPTH_OTHER = 64, 8

    def merge(dst, src_):
        for k, v in src_.items():
            if dst.get(k, -1) < v:
                dst[k] = v

    def implies(k, s, v):
        return k.get(s, -1) >= v

    cum = {}          # sem -> cumulative increments so far (schedule order)
    snap = {}         # sem -> list of (post_cum, completion-knowledge dict)
    kw = {}           # engine -> knowledge from dispatch-blocking waits
    kc = {}           # engine -> knowledge from >=Q-back completions
    ring = {}         # engine -> list of (own-increments dict)

    def snap_lookup(s, v):
        lst = snap.get(s)
        if not lst:
            return {}
        lo, hi = 0, len(lst)
        while lo < hi:
            mid = (lo + hi) // 2
            if lst[mid][0] >= v:
                hi = mid
            else:
                lo = mid + 1
        return lst[lo][1] if lo < len(lst) else {}

    for b in nc.m.functions[0].blocks:
        for inst in b.instructions:
            si = inst.sync_info
            eng = str(inst.engine)
            opcode = type(inst).__name__
            is_dma = ("DMACopy" in opcode or "TriggeredCopy" in opcode
                      or "Collective" in opcode)
            waits = list(si.on_wait or []) if si is not None else []
            updates = list(si.on_update or []) if si is not None else []

            is_pool = "Pool" in eng
            if is_dma:
                kdisp = {}
            else:
                if not is_pool:
                    q = QDEPTH_PE if "PE" in eng else QDEPTH_OTHER
                    r = ring.setdefault(eng, [])
                    if len(r) >= q:
                        merge(kc.setdefault(eng, {}), r.pop(0))
                kdisp = dict(kw.get(eng, {}))
                merge(kdisp, kc.get(eng, {}))

            wknow = []
            for w in waits:
                g = dict(snap_lookup(w.ant_name, w.wait_value))
                if g.get(w.ant_name, -1) < w.wait_value:
                    g[w.ant_name] = w.wait_value
                wknow.append(g)

            kept = list(range(len(waits)))
            if len(waits) > 1:
                changed = True
                while changed and len(kept) > 1:
                    changed = False
                    for idx in list(kept):
                        w = waits[idx]
                        if is_pool and "Pool" in w.ant_name:
                            k_union = {}
                            for j in kept:
                                if j != idx:
                                    merge(k_union, wknow[j])
                        else:
                            k_union = dict(kdisp)
                            for j in kept:
                                if j != idx:
                                    merge(k_union, wknow[j])
                        if implies(k_union, w.ant_name, w.wait_value):
                            kept.remove(idx)
                            changed = True
                            break
                if len(kept) < len(waits):
                    inst.sync_info = mybir.SyncInfo(
                        on_wait=[waits[i] for i in kept], on_update=updates)

            k_wait = dict(kdisp)
            for g in wknow:
                merge(k_wait, g)

            own_incs = {}
            for u in updates:
                s = u.ant_name
                cum[s] = cum.get(s, 0) + u.update_value
                own_incs[s] = cum[s]

            if own_incs:
                comp = dict(k_wait)
                merge(comp, own_incs)
                for s, v in own_incs.items():
                    snap.setdefault(s, []).append((v, comp))

            if not is_dma:
                merge(kw.setdefault(eng, {}), k_wait)
                ring.setdefault(eng, []).append(own_incs)


def get_program():
    global _PROGRAM
    if _PROGRAM is None:
        _PROGRAM = _build_program()
    return _PROGRAM


def fold_weights(inputs, fi, fj):
    """Host-side weight transform -> dict of bf16 arrays in kernel layout.
    fi/fj: flip the kernel along H/W; stride-2 convs get their flipped 3x3
    placed at offset (fi,fj) of a 4x4 window (1x1 ds: 2x2 window) to fix
    the stride-2 sampling phase under flip."""
    out = {}
    for n in W3_NAMES + W16_NAMES:
        w = np.asarray(inputs[n], np.float32) * W_SCALE[n]   # [O,I,3,3]
        if fi:
            w = w[:, :, ::-1, :]
        if fj:
            w = w[:, :, :, ::-1]
        if n in W16_NAMES:
            w4 = np.zeros((128, 128, 4, 4), np.float32)
            w4[:, :, fi: fi + 3, fj: fj + 3] = w
            out[n] = np.ascontiguousarray(
                w4.transpose(1, 2, 3, 0).reshape(128, 16, 128)).astype(BF16)
        else:
            out[n] = np.ascontiguousarray(
                w.transpose(1, 2, 3, 0).reshape(128, 9, 128)).astype(BF16)
    for n in ('e1b0ds', 'e2b0ds'):
        w = np.asarray(inputs[n], np.float32)[:, :, 0, 0] * W_SCALE[n]
        w2 = np.zeros((128, 128, 2, 2), np.float32)
        w2[:, :, fi, fj] = w
        out[n] = np.ascontiguousarray(
            w2.transpose(1, 2, 3, 0).reshape(128, 4, 128)).astype(BF16)
    for n in ('dec0w', 'dec1w'):
        w = np.asarray(inputs[n], np.float32) * W_SCALE[n]   # [I,O,2,2]
        if fi:
            w = w[:, :, ::-1, :]
        if fj:
            w = w[:, :, :, ::-1]
        out[n] = np.ascontiguousarray(
            w.transpose(0, 2, 3, 1).reshape(128, 4, 128)).astype(BF16)
    return out


def make_in_maps(inputs):
    x = np.asarray(inputs['x'], np.float32)
    wpacks = {}
    for fi in range(2):
        for fj in range(2):
            folded = fold_weights(inputs, fi, fj)
            wp = np.concatenate(
                [folded[n].reshape(128, -1) for n, _, _ in WPACK_OFFS],
                axis=1)
            assert wp.shape == (128, WPACK_LEN)
            wpacks[(fi, fj)] = wp
    Pimg = np.pad(x, ((0, 0), (0, 0), (1, 1), (1, 1)))
    in_maps = []
    for b in range(2):
        for i in range(2):
            for j in range(2):
                rs, cs = RS2[i], RS2[j]
                xt = Pimg[b, :, rs: rs + B0, cs: cs + B0]
                if i:
                    xt = xt[:, ::-1, :]
                if j:
                    xt = xt[:, :, ::-1]
                xt = np.ascontiguousarray(xt).astype(BF16)
                in_maps.append({'xt': xt, 'wpack': wpacks[(i, j)]})
    return in_maps


def assemble(outs):
    """outs: list of 8 dicts with 'out' [128,128,128] -> [2,128,256,256]."""
    res = np.zeros((2, 128, 256, 256), np.float32)
    idx = 0
    for b in range(2):
        for i in range(2):
            for j in range(2):
                o = np.asarray(outs[idx]['out'], np.float32)
                if i:
                    o = o[:, ::-1, :]
                if j:
                    o = o[:, :, ::-1]
                res[b, :, OWN[i]: OWN[i] + 128, OWN[j]: OWN[j] + 128] = o
                idx += 1
    return res


def run_spmd(inputs, **kwargs):
    from concourse.bass_utils import run_bass_kernel_spmd
    nc = get_program()
    in_maps = make_in_maps(inputs)
    res = run_bass_kernel_spmd(nc, in_maps, core_ids=list(range(8)), **kwargs)
    return res


def kernel(**inputs):
    res = run_spmd(inputs)
    return assemble(res.results)


def bench_exec(inputs, iters=20, warmup=3):
    """Time on-device execution by pipelining async dispatches.

    Replicates bass2jax.run_bass_via_pjrt's shard_map execution, pre-places
    inputs on the 8 devices, and chains donation so repeated executions
    queue back-to-back on the devices. Returns (ns_per_iter, outputs)."""
    import time
    import jax
    from jax.sharding import Mesh, PartitionSpec, NamedSharding
    from jax.experimental.shard_map import shard_map
    import concourse.mybir as mybir
    from concourse.bass2jax import (
        _bass_exec_p, install_neuronx_cc_hook, partition_id_tensor)

    install_neuronx_cc_hook()
    nc = get_program()
    in_maps = make_in_maps(inputs)
    n_cores = len(in_maps)
    partition_name = (nc.partition_id_tensor.name
                      if nc.partition_id_tensor else None)

    in_names, out_names, out_avals, zero_outs = [], [], [], []
    for alloc in nc.m.functions[0].allocations:
        if not isinstance(alloc, mybir.MemoryLocationSet):
            continue
        name = alloc.memorylocations[0].name
        if alloc.kind == "ExternalInput":
            if name != partition_name:
                in_names.append(name)
        elif alloc.kind == "ExternalOutput":
            out_names.append(name)
            shape = tuple(alloc.tensor_shape)
            dtype = mybir.dt.np(alloc.dtype)
            out_avals.append(jax.core.ShapedArray(shape, dtype))
            zero_outs.append(np.zeros(shape, dtype))
    n_params = len(in_names)
    n_outs = len(out_avals)
    in_names_all = in_names + out_names
    if partition_name is not None:
        in_names_all = in_names_all + [partition_name]

    def _body(*args):
        operands = list(args)
        if partition_name is not None:
            operands.append(partition_id_tensor())
        outs = _bass_exec_p.bind(
            *operands,
            out_avals=tuple(out_avals),
            in_names=tuple(in_names_all),
            out_names=tuple(out_names),
            lowering_input_output_aliases=(),
            sim_require_finite=True,
            sim_require_nnan=True,
            nc=nc,
        )
        return tuple(outs)

    devices = jax.devices()[:n_cores]
    mesh = Mesh(np.asarray(devices), ("core",))
    spec = PartitionSpec("core")
    donate = tuple(range(n_params, n_params + n_outs))
    f = jax.jit(
        shard_map(_body, mesh=mesh, in_specs=(spec,) * (n_params + n_outs),
                  out_specs=(spec,) * n_outs, check_rep=False),
        donate_argnums=donate, keep_unused=True)

    sharding = NamedSharding(mesh, spec)
    dev_ins = [
        jax.device_put(
            np.concatenate([np.asarray(m[name]) for m in in_maps], axis=0),
            sharding)
        for name in in_names]
    outs = tuple(
        jax.device_put(np.concatenate([z] * n_cores, axis=0), sharding)
        for z in zero_outs)

    for _ in range(warmup):
        outs = f(*dev_ins, *outs)
    jax.block_until_ready(outs)

    def window(n):
        nonlocal outs
        t0 = time.perf_counter()
        for _ in range(n):
            outs = f(*dev_ins, *outs)
        jax.block_until_ready(outs)
        return time.perf_counter() - t0

    if iters >= 60:
        n1 = iters // 4
        t1 = min(window(n1), window(n1))
        t2 = min(window(iters), window(iters))
        ns = (t2 - t1) / (iters - n1) * 1e9
    else:
        ns = window(iters) / iters * 1e9
    return ns, outs


# revision 27
# speedup vs baseline: 1.3198x; 1.3198x over previous
"""Trainium2 Bass kernel for nn_CascadeDEDBackbone (ResNet-style encoder/decoder,
[2,128,256,256] f32, all convs 128->128ch).

Strategy (self-contained, hardcoded):
  - 8 cores = batch(2) x H-half(2) x W-half(2). Host flip-normalizes every
    tile so its owned 128x128 anchors at local (0,0) (weights flipped per
    core to compensate); each conv computes only the region later stages
    consume (validity shrinks 1px/conv toward the high side).
  - Stride-2 convs are not flip-equivariant (sampling phase), so they use a
    4x4 tap window (16 matmuls) with each core's flipped 3x3 weights placed
    at offset (i,j) by the host; 1x1 downsamples use a 2x2 window. This
    keeps one SPMD program for all 8 cores.
  - Input margin is 7px (vs 24 fully-redundant): two halo exchanges refill
    margins mid-net via three CONCURRENT 2-rank AllReduces (H-pair rows,
    W-pair cols, diagonal-pair corner; +local subtract, exact in f32):
    exchange A after e1b0 (margin 2 at 128-res), exchange B on f1
    (margin 7). Producers emit boundary chunks first and consumers emit
    all interior chunks before any margin strip (the PE stream executes
    in emission order), so the collectives hide under interior compute.
    Exchange DMAs ride gpsimd/SWDGE lanes to dodge the contended HWDGE
    ring-reuse waits (one-wait ISA limit).
  - On-core: a KxK conv = K^2 accumulated 128x128 matmuls over shifted APs
    (channels = partitions). PSUM accumulates f32; VectorE evacuates with
    fused relu/scale/residual-add; activations stay resident in SBUF.
  - Decoder evacuation alternates per chunk between DVE and an ACT->Pool
    chain so the deconv stages stay PE-bound; output is bf16 128x128/core.
"""

import os
import sys

import numpy as np
import ml_dtypes

for _p in ("/opt/trn_rl_repo", "/opt/trn_rl_repo/concourse"):
    if os.path.isdir(_p) and _p not in sys.path:
        sys.path.insert(0, _p)

BF16 = ml_dtypes.bfloat16
BN_S = float(1.0 / np.sqrt(1.0 + 1e-3))

# region geometry: owned 128, host margin 7 (incl 1-px ring -> 136 input)
M0 = 7
B0 = 136
REG_L0 = [134, 133, 132, 131]   # e0b0c1, e0b0c2, e0b1c1, e0b1c2(f0)
REG_L1 = [65, 64, 65, 64]       # e1b0c1, e1b0c2, e1b1c1, e1b1c2(f1)
REG_L2 = [35, 34, 33, 32]       # e2b0c1, e2b0c2, e2b1c1, e2b1c2(e2o)
MA = 2                          # exchange A margin (bo1, 128-res)
MB = 7                          # exchange B margin (f1p, 128-res)
DEC0_IN = 32
DEC1_IN = 64
OWNED = 128
RS2 = [0, 122]                  # host slice start per tile index
OWN = [0, 128]

OFF9 = [(dy, dx) for dy in range(3) for dx in range(3)]
OFF16 = [(dy, dx) for dy in range(4) for dx in range(4)]
OFF4 = [(k, l) for k in range(2) for l in range(2)]

GRP_H = [[0, 2], [1, 3], [4, 6], [5, 7]]
GRP_W = [[0, 1], [2, 3], [4, 5], [6, 7]]
GRP_D = [[0, 3], [1, 2], [4, 7], [5, 6]]

W3_NAMES = ['e0b0c1', 'e0b0c2', 'e0b1c1', 'e0b1c2',
            'e1b0c2', 'e1b1c1', 'e1b1c2',
            'e2b0c2', 'e2b1c1', 'e2b1c2']
W16_NAMES = ['e1b0c1', 'e2b0c1']
# BN-scale folding: weights consuming pre-scaled f0'/f1' buffers get their
# BN fold cancelled.
W_SCALE = {'e0b0c1': BN_S, 'e0b0c2': BN_S, 'e0b1c1': BN_S, 'e0b1c2': BN_S,
           'e1b0c1': 1.0, 'e1b0c2': BN_S, 'e1b0ds': 1.0,
           'e1b1c1': BN_S, 'e1b1c2': BN_S,
           'e2b0c1': 1.0, 'e2b0c2': BN_S, 'e2b0ds': 1.0,
           'e2b1c1': BN_S, 'e2b1c2': BN_S,
           'dec0w': BN_S * BN_S, 'dec1w': BN_S * BN_S}

_W_ORDER = ['e0b0c1', 'e0b0c2', 'e0b1c1', 'e0b1c2',
            'e1b0c1', 'e1b0c2', 'e1b0ds', 'e1b1c1', 'e1b1c2',
            'e2b0c1', 'e2b0c2', 'e2b0ds', 'e2b1c1', 'e2b1c2',
            'dec0w', 'dec1w']
_W_LENS = {**{n: 9 * 128 for n in W3_NAMES},
           **{n: 16 * 128 for n in W16_NAMES},
           'e1b0ds': 4 * 128, 'e2b0ds': 4 * 128,
           'dec0w': 4 * 128, 'dec1w': 4 * 128}
WPACK_OFFS = []
_off = 0
for _n in _W_ORDER:
    WPACK_OFFS.append((_n, _off, _W_LENS[_n]))
    _off += _W_LENS[_n]
WPACK_LEN = _off

_PROGRAM = None  # cached bass.Bass


def _build_program(n_repeat=None):
    import concourse.bass as bass
    import concourse.mybir as mybir
    import concourse.tile as tile
    from contextlib import ExitStack

    if n_repeat is None:
        n_repeat = int(os.environ.get("K_FULL_REPEAT", "1"))
    use_cc = os.environ.get("K_NO_CC", "0") != "1"

    bf = mybir.dt.bfloat16
    f32 = mybir.dt.float32
    ADD = mybir.AluOpType.add
    SUB = mybir.AluOpType.subtract
    MAX = mybir.AluOpType.max
    MULT = mybir.AluOpType.mult

    nc = bass.Bass()

    xt_d = nc.dram_tensor("xt", [128, B0, B0], bf, kind="ExternalInput")
    wpack_d = nc.dram_tensor("wpack", [128, WPACK_LEN], bf, kind="ExternalInput")
    out_d = nc.dram_tensor("out", [128, OWNED, OWNED], bf,
                           kind="ExternalOutput")

    with tile.TileContext(nc) as tc, ExitStack() as ctx:
        wp = ctx.enter_context(tc.tile_pool(name="wpool", bufs=1))
        wslab = wp.tile([128, WPACK_LEN], bf, tag="wpack", name="wslab")
        w3 = {}
        for n, off, ln in WPACK_OFFS:
            view = wslab[:, off: off + ln]
            w3[n] = view.rearrange("c (n m) -> c n m", n=ln // 128)

        def dma_w(n):
            _, off, ln = next(t for t in WPACK_OFFS if t[0] == n)
            nc.sync.dma_start(wslab[:, off: off + ln],
                              wpack_d[:, off: off + ln])

        pers = ctx.enter_context(tc.tile_pool(name="pers", bufs=1))

        psp = ctx.enter_context(
            tc.tile_pool(name="psp", bufs=4, space=bass.MemorySpace.PSUM))
        tmpp = ctx.enter_context(tc.tile_pool(name="tmpp", bufs=8))
        dramp = ctx.enter_context(
            tc.tile_pool(name="dramp", bufs=1, space="DRAM"))
        xch = ctx.enter_context(tc.tile_pool(name="xch", bufs=1))

        # DMA order: first conv's weights, then the input in row bands (so
        # the first conv chunks start as soon as their rows land), then the
        # remaining weights. After each band, a tiny DVE read (absorber)
        # publishes the band's DMA completion into DVE's vector clock.
        _, _w0off, _ = next(t for t in WPACK_OFFS if t[0] == 'e0b0c1')
        nc.sync.dma_start(wslab[:, _w0off: _w0off + 128],
                          wpack_d[:, _w0off: _w0off + 128])
        nc.sync.dma_start(wslab[:, _w0off + 128: _w0off + 9 * 128],
                          wpack_d[:, _w0off + 128: _w0off + 9 * 128])

        def dma_input(xt):
            bands = [0, 3, 6] + list(range(26, B0, 20)) + [B0]
            for a, b_hi in zip(bands[:-1], bands[1:]):
                nc.sync.dma_start(xt[:, a:b_hi, :], xt_d[:, a:b_hi, :])
                scratch = tmpp.tile([128, 1, 1], bf, tag="scratch",
                                    name="scratch")
                nc.vector.tensor_copy(scratch[:], xt[:, b_hi - 1:b_hi, 0:1])

        xt = pers.tile([128, B0, B0], bf, tag="pers", name="xt_s")
        dma_input(xt)
        _w_emit = ['dec1w', 'dec0w', 'e2b0ds', 'e2b1c2', 'e2b1c1', 'e2b0c2',
                   'e2b0c1', 'e1b0ds', 'e1b1c2', 'e1b1c1', 'e1b0c2', 'e1b0c1',
                   'e0b1c2', 'e0b1c1', 'e0b0c2']
        assert set(_w_emit) == {n for n, _, _ in WPACK_OFFS} - {'e0b0c1'}
        for n in _w_emit:
            dma_w(n)
        del dma_w

        def ring_zero(t, H):
            # only the low-side ring (image-edge zero padding) is read
            nc.vector.memset(t[:, 0, :], 0.0)
            nc.vector.memset(t[:, 1:, 0], 0.0)

        def conv3x3(src, dst, w, stride, rchunk, y_off=0, hr=None, x0=0,
                    cw=None, reg=None, scale=1.0, resid=None, extra_mm=None,
                    taps=OFF9):
            """Compute out rows y_off..y_off+hr-1, cols x0..x0+cw-1
            (0-based within interior; dst row = 1+y). reg: full region (for
            defaults). resid: callable (y0, rr, x0, cw) -> identity AP."""
            hr = reg - y_off if hr is None else hr
            cw = reg - x0 if cw is None else cw
            if resid is not None:
                ab = tmpp.tile([128, 1, cw], bf, tag="absorb", name="absorb")
                nc.vector.tensor_copy(ab[:], resid(y_off + hr - 1, 1, x0, cw))
            n_ext = 0 if extra_mm is None else 4
            last = len(taps) + n_ext - 1
            for y0 in range(y_off, y_off + hr, rchunk):
                rr = min(rchunk, y_off + hr - y0)
                ps = psp.tile([128, rr, cw], f32, tag="ps", name="ps")
                for o, (dy, dx) in enumerate(taps):
                    r_lo = y0 * stride + dy
                    c_lo = x0 * stride + dx
                    rhs = src[:, r_lo: r_lo + (rr - 1) * stride + 1: stride,
                              c_lo: c_lo + (cw - 1) * stride + 1: stride]
                    nc.tensor.matmul(ps[:], w[:, o, :], rhs,
                                     start=(o == 0), stop=(o == last))
                if extra_mm is not None:
                    w4, src2 = extra_mm
                    for q, (dy, dx) in enumerate(OFF4):
                        rhs = src2[:, 1 + 2 * y0 + dy:
                                   1 + 2 * y0 + dy + (rr - 1) * 2 + 1: 2,
                                   1 + 2 * x0 + dx:
                                   1 + 2 * x0 + dx + (cw - 1) * 2 + 1: 2]
                        nc.tensor.matmul(ps[:], w4[:, q, :], rhs,
                                         start=False,
                                         stop=(len(taps) + q == last))
                dst_ap = dst[:, 1 + y0: 1 + y0 + rr, 1 + x0: 1 + x0 + cw]
                if resid is None:
                    nc.vector.tensor_scalar(dst_ap, ps[:], 0.0, scale, MAX,
                                            MULT)
                else:
                    tmp = tmpp.tile([128, rr, cw], bf, tag="tmpr", name="tmpr")
                    nc.vector.tensor_tensor(tmp[:], ps[:],
                                            resid(y0, rr, x0, cw), op=ADD)
                    nc.vector.tensor_scalar(dst_ap, tmp[:], 0.0, scale, MAX,
                                            MULT)

        def interior(buf):
            return lambda y0, rr, x0, cw: buf[:, 1 + y0: 1 + y0 + rr,
                                              1 + x0: 1 + x0 + cw]

        def _ar_chain(ox, shp, grp, sfx, tag):
            ci = dramp.tile(shp, f32, tag=f"ci{sfx}{tag}")
            co = dramp.tile(shp, f32, tag=f"co{sfx}{tag}")
            nc.gpsimd.dma_start(ci[:], ox[:])
            if use_cc:
                nc.gpsimd.collective_compute(
                    "AllReduce", ADD, replica_groups=grp,
                    ins=[ci.opt()], outs=[co.opt()])
            else:
                nc.sync.dma_start(co[:], ci[:])
            return co

        def exchange_hd(buf, n, m, tag):
            """Early phase: launch the H-row and diagonal-corner AllReduces.
            Their payloads need only the producer's BOUNDARY chunk (emitted
            first), so issuing here - before the interior conv evacs enter
            the DVE stream - gives the collectives the whole interior
            compute as flight time. SPMD-safe via AllReduce(add) of f32 +
            local subtract (exact)."""
            oxh = xch.tile([128, m, n], f32, tag=f"oxh{tag}", name="oxh")
            for r in range(m):
                nc.vector.tensor_copy(oxh[:, r, :], buf[:, n - r, 1:1 + n])
            oxd = xch.tile([128, m, m], f32, tag=f"oxd{tag}", name="oxd")
            for r in range(m):
                nc.vector.tensor_copy(oxd[:, r, :],
                                      buf[:, n - r, n - m + 1: n + 1])
            coh = _ar_chain(oxh, [128, m, n], GRP_H, "h", tag)
            cod = _ar_chain(oxd, [128, m, m], GRP_D, "d", tag)
            # apply H and corner margins HERE, on the Pool engine: Pool is
            # idle during the exchange, so the (long-latency) subtracts do
            # not head-of-line-block the DVE stream's interior evacs, and
            # the H margins are available as soon as the AR lands.
            inh = xch.tile([128, m, n], f32, tag=f"inh{tag}", name="inh")
            nc.gpsimd.dma_start(inh[:], coh[:])
            nc.gpsimd.tensor_tensor(buf[:, n + 1: n + 1 + m, 1: 1 + n],
                                    inh[:], oxh[:], op=SUB)
            ind = xch.tile([128, m, m], f32, tag=f"ind{tag}", name="ind")
            nc.gpsimd.dma_start(ind[:], cod[:])
            for c in range(m):
                nc.gpsimd.tensor_tensor(
                    buf[:, n + 1: n + 1 + m, n + 1 + c],
                    ind[:, :, m - 1 - c], oxd[:, :, m - 1 - c], op=SUB)

        def exchange_w(buf, n, m, tag):
            """Late phase: W-col AllReduce (needs every row chunk of buf),
            then apply the W margins. W outbox stored TRANSPOSED [m, n]
            so the bounce DMAs burst well (28B inner runs DMA ~5x slower).
            Writes rows 1..n only - disjoint from the corner applied in
            exchange_hd."""
            oxw = xch.tile([128, m, n], f32, tag=f"oxw{tag}", name="oxw")
            for c in range(m):
                nc.vector.tensor_copy(oxw[:, c, :], buf[:, 1: 1 + n, n - c])
            cow = _ar_chain(oxw, [128, m, n], GRP_W, "w", tag)
            inw = xch.tile([128, m, n], f32, tag=f"inw{tag}", name="inw")
            nc.gpsimd.dma_start(inw[:], cow[:])
            for c in range(m):
                nc.vector.tensor_tensor(buf[:, 1: 1 + n, n + 1 + c],
                                        inw[:, c, :], oxw[:, c, :], op=SUB)

        def body(rep, xt):
            rtag = f"r{rep}" if rep else ""
            # ------------- level 0 (256-res, regions 134..131) -----------
            with tc.tile_pool(name=f"work{rtag}", bufs=2) as workp:
                c1 = workp.tile([128, B0, B0], bf, tag="work", name="b0c1")
                ring_zero(c1, B0 - 2)
                for _rep in range(int(os.environ.get("K_CAL_REPEAT", "1"))):
                    conv3x3(xt, c1, w3['e0b0c1'], 1, 3, reg=REG_L0[0])
                b0 = workp.tile([128, B0, B0], bf, tag="work", name="b0out")
                ring_zero(b0, B0 - 2)
                conv3x3(c1, b0, w3['e0b0c2'], 1, 3, reg=REG_L0[1],
                        resid=interior(xt))
                c1 = workp.tile([128, B0, B0], bf, tag="work", name="b1c1")
                ring_zero(c1, B0 - 2)
                conv3x3(b0, c1, w3['e0b1c1'], 1, 3, reg=REG_L0[2])
                f0p = pers.tile([128, B0, B0], bf, tag="pers",
                                name="f0p")
                ring_zero(f0p, B0 - 2)
                conv3x3(c1, f0p, w3['e0b1c2'], 1, 3, reg=REG_L0[3],
                        scale=BN_S, resid=interior(b0))

            # ------------- levels 1+2 / decoder pools -------------
            with tc.tile_pool(name=f"acts2{rtag}", bufs=1) as acts2, \
                 tc.tile_pool(name=f"lv{rtag}", bufs=1) as lv, \
                 tc.tile_pool(name=f"ostp{rtag}", bufs=6) as ostp, \
                 tc.tile_pool(name=f"tmpa{rtag}", bufs=4) as tmpa, \
                 tc.tile_pool(name=f"dpsp{rtag}", bufs=4,
                              space=bass.MemorySpace.PSUM) as dpsp:

                # ---- level 1 block 0 (stride-2, 16-tap window) ----
                c1 = lv.tile([128, 67, 67], bf, tag="c1", name="c1")
                ring_zero(c1, 65)
                conv3x3(f0p, c1, w3['e1b0c1'], 2, 6, reg=REG_L1[0],
                        taps=OFF16)
                bo1 = lv.tile([128, 67, 67], bf, tag="bo1", name="bo1")
                ring_zero(bo1, 65)
                # boundary rows first so exchange A starts early
                conv3x3(c1, bo1, w3['e1b0c2'], 1, 6, y_off=60, hr=4,
                        reg=REG_L1[1], extra_mm=(w3['e1b0ds'], f0p))
                exchange_hd(bo1, 64, MA, f"A{rtag}")
                conv3x3(c1, bo1, w3['e1b0c2'], 1, 6, y_off=0, hr=60,
                        reg=REG_L1[1], extra_mm=(w3['e1b0ds'], f0p))
                exchange_w(bo1, 64, MA, f"A{rtag}")

                # ---- level 1 block 1 (interior + margin strips) ----
                c1b = lv.tile([128, 67, 67], bf, tag="b1c1", name="b1c1")
                ring_zero(c1b, 65)
                conv3x3(bo1, c1b, w3['e1b1c1'], 1, 6, y_off=0, hr=63,
                        x0=0, cw=63, reg=REG_L1[2])
                conv3x3(bo1, c1b, w3['e1b1c1'], 1, 63, y_off=0, hr=63,
                        x0=63, cw=2, reg=REG_L1[2])
                conv3x3(bo1, c1b, w3['e1b1c1'], 1, 6, y_off=63, hr=2,
                        x0=0, cw=65, reg=REG_L1[2])
                f1p = acts2.tile([128, 72, 72], bf, tag="f1p", name="f1p")
                ring_zero(f1p, 70)
                conv3x3(c1b, f1p, w3['e1b1c2'], 1, 6, y_off=54, hr=10,
                        reg=REG_L1[3], scale=BN_S, resid=interior(bo1))
                exchange_hd(f1p, 64, MB, f"B{rtag}")
                conv3x3(c1b, f1p, w3['e1b1c2'], 1, 6, y_off=0, hr=54,
                        reg=REG_L1[3], scale=BN_S, resid=interior(bo1))
                exchange_w(f1p, 64, MB, f"B{rtag}")

                # ---- level 2 ----
                # Emit ALL interiors first, then all margin strips: the PE
                # stream runs in emission order, so strips (which wait on
                # exchange B) must sit behind every interior chunk to avoid
                # head-of-line blocking while the collectives fly.
                c2 = lv.tile([128, 37, 37], bf, tag="c1", name="c2")
                ring_zero(c2, 35)
                bo2 = lv.tile([128, 36, 36], bf, tag="bo1", name="bo2")
                ring_zero(bo2, 34)
                c2b = lv.tile([128, 35, 35], bf, tag="b1c1", name="b2c1")
                ring_zero(c2b, 33)
                e2o = acts2.tile([128, 34, 34], bf, tag="e2o", name="e2o")
                ring_zero(e2o, 32)
                conv3x3(f1p, c2, w3['e2b0c1'], 2, 13, y_off=0, hr=31,
                        x0=0, cw=31, reg=REG_L2[0], taps=OFF16)
                conv3x3(c2, bo2, w3['e2b0c2'], 1, 13, y_off=0, hr=30,
                        x0=0, cw=30, reg=REG_L2[1],
                        extra_mm=(w3['e2b0ds'], f1p))
                conv3x3(bo2, c2b, w3['e2b1c1'], 1, 13, y_off=0, hr=29,
                        x0=0, cw=29, reg=REG_L2[2])
                conv3x3(c2b, e2o, w3['e2b1c2'], 1, 13, y_off=0, hr=28,
                        x0=0, cw=28, reg=REG_L2[3], resid=interior(bo2))
                # EARLY strips: bottom-left portions read only the H
                # margin, applied early on Pool by exchange_hd - they fill
                # the W-AR window before the W/corner-dependent strips.
                conv3x3(f1p, c2, w3['e2b0c1'], 2, 13, y_off=31, hr=4,
                        x0=0, cw=31, reg=REG_L2[0], taps=OFF16)
                conv3x3(c2, bo2, w3['e2b0c2'], 1, 13, y_off=30, hr=4,
                        x0=0, cw=30, reg=REG_L2[1],
                        extra_mm=(w3['e2b0ds'], f1p))
                conv3x3(bo2, c2b, w3['e2b1c1'], 1, 13, y_off=29, hr=4,
                        x0=0, cw=29, reg=REG_L2[2])
                conv3x3(c2b, e2o, w3['e2b1c2'], 1, 13, y_off=28, hr=4,
                        x0=0, cw=28, reg=REG_L2[3], resid=interior(bo2))
                # LATE strips (need W/corner margins), cascade order
                conv3x3(f1p, c2, w3['e2b0c1'], 2, 31, y_off=0, hr=31,
                        x0=31, cw=4, reg=REG_L2[0], taps=OFF16)
                conv3x3(f1p, c2, w3['e2b0c1'], 2, 13, y_off=31, hr=4,
                        x0=31, cw=4, reg=REG_L2[0], taps=OFF16)
                conv3x3(c2, bo2, w3['e2b0c2'], 1, 30, y_off=0, hr=30,
                        x0=30, cw=4, reg=REG_L2[1],
                        extra_mm=(w3['e2b0ds'], f1p))
                conv3x3(c2, bo2, w3['e2b0c2'], 1, 13, y_off=30, hr=4,
                        x0=30, cw=4, reg=REG_L2[1],
                        extra_mm=(w3['e2b0ds'], f1p))
                conv3x3(bo2, c2b, w3['e2b1c1'], 1, 29, y_off=0, hr=29,
                        x0=29, cw=4, reg=REG_L2[2])
                conv3x3(bo2, c2b, w3['e2b1c1'], 1, 13, y_off=29, hr=4,
                        x0=29, cw=4, reg=REG_L2[2])
                conv3x3(c2b, e2o, w3['e2b1c2'], 1, 28, y_off=0, hr=28,
                        x0=28, cw=4, reg=REG_L2[3], resid=interior(bo2))
                conv3x3(c2b, e2o, w3['e2b1c2'], 1, 13, y_off=28, hr=4,
                        x0=28, cw=4, reg=REG_L2[3], resid=interior(bo2))

                # ---------------- decoder ----------------
                RELU = mybir.ActivationFunctionType.Relu
                COPY = mybir.ActivationFunctionType.Copy

                pool_px = [None]

                def act_absorb():
                    if pool_px[0] is not None:
                        ab = tmpa.tile([128, 1], bf, tag="actab", name="actab")
                        nc.scalar.activation(ab[:], pool_px[0], COPY)

                # stage 0: y1 = BN_S*relu(deconv(e2o, dec0w*s)) + f1p
                D0, D1 = DEC0_IN, DEC1_IN
                y1 = acts2.tile([128, D1, D1], bf, tag="y1", name="y1")
                use_chain = os.environ.get("K_DEC_CHAIN", "1") == "1"
                for ci, y0 in enumerate(range(0, D0, 9)):
                    chain = (ci % 2 == 1) and use_chain
                    if chain:
                        act_absorb()
                    for p, (k, l) in enumerate(OFF4):
                        rr = min(9, D0 - y0)
                        pool = psp if p % 2 == 0 else dpsp
                        tag = "ps" if p % 2 == 0 else "dps"
                        ps = pool.tile([128, rr, D0], f32, tag=tag, name="ps")
                        nc.tensor.matmul(ps[:], w3['dec0w'][:, p, :],
                                         e2o[:, 1 + y0: 1 + y0 + rr, 1: 1 + D0])
                        y1_ap = y1[:, 2 * y0 + k: 2 * (y0 + rr - 1) + k + 1: 2,
                                   l: 2 * (D0 - 1) + l + 1: 2]
                        f1_ap = f1p[:, 1 + 2 * y0 + k:
                                    1 + 2 * (y0 + rr - 1) + k + 1: 2,
                                    1 + l: 1 + 2 * (D0 - 1) + l + 1: 2]
                        # s^2 folded into dec0w: relu(s^2 psum) = s relu(s psum)
                        if not chain:
                            nc.vector.scalar_tensor_tensor(
                                y1_ap, ps[:], 0.0, f1_ap, op0=MAX, op1=ADD)
                        else:
                            tmp = tmpa.tile([128, rr, D0], bf, tag="t0",
                                            name="t0", bufs=8)
                            nc.scalar.activation(tmp[:], ps[:], RELU)
                            if p == 0:
                                pab = tmpa.tile([128, 1], bf,
                                                tag=f"pab0_{ci}{rtag}",
                                                name="pab", bufs=1)
                                nc.gpsimd.tensor_copy(pab[:], tmp[:, 0, 0:1])
                            nc.gpsimd.tensor_tensor(y1_ap, tmp[:], f1_ap,
                                                    op=ADD)
                            pool_px[0] = y1[:, 2 * y0 + k: 2 * y0 + k + 1,
                                            l: l + 1]

                # stage 1: out = BN_S*relu(deconv(y1, dec1w*s)) + f0p
                ost_tiles = []
                SUB6 = 6
                nch = (D1 + SUB6 - 1) // SUB6

                def ensure_ost(i):
                    while len(ost_tiles) <= min(i, nch - 1):
                        j = len(ost_tiles)
                        rrj = min(SUB6, D1 - j * SUB6)
                        t = ostp.tile([128, 2 * rrj, OWNED], bf, tag="ost",
                                      name="ost")
                        ost_tiles.append(t)
                        if j % 2 == 0:
                            nc.vector.memset(t[:, 0:1, 0:1], 0.0)
                        else:
                            nc.gpsimd.memset(t[:, 0:1, 0:1], 0.0)

                for ci, y0 in enumerate(range(0, D1, SUB6)):
                    ensure_ost(ci + 4)
                    chain = (ci % 2 == 1) and use_chain
                    rr = min(SUB6, D1 - y0)
                    ost = ost_tiles[ci]
                    if chain:
                        act_absorb()
                    for p, (k, l) in enumerate(OFF4):
                        ps = dpsp.tile([128, rr * D1], f32, tag="dps",
                                       name="dps")
                        nc.tensor.matmul(ps[:], w3['dec1w'][:, p, :],
                                         y1[:, y0: y0 + rr, :])
                        f0_ap = f0p[:, 1 + 2 * y0 + k:
                                    1 + 2 * (y0 + rr - 1) + k + 1: 2,
                                    1 + l: 1 + 2 * (D1 - 1) + l + 1: 2]
                        ost_ap = ost[:, k: 2 * (rr - 1) + k + 1: 2,
                                     l: 2 * (D1 - 1) + l + 1: 2]
                        ps_2d = ps[:].rearrange("c (r w) -> c r w", r=rr)
                        # s^2 folded into dec1w
                        if not chain:
                            nc.vector.scalar_tensor_tensor(
                                ost_ap, ps_2d, 0.0, f0_ap, op0=MAX, op1=ADD)
                        else:
                            tmp = tmpa.tile([128, rr * D1], bf, tag="t1",
                                            name="t1", bufs=8)
                            nc.scalar.activation(tmp[:], ps[:], RELU)
                            if p == 0:
                                pab = tmpa.tile([128, 1], bf,
                                                tag=f"pab1_{ci}{rtag}",
                                                name="pab", bufs=1)
                                nc.gpsimd.tensor_copy(pab[:], tmp[:, 0:1])
                            tmp_2d = tmp[:].rearrange("c (r w) -> c r w", r=rr)
                            nc.gpsimd.tensor_tensor(ost_ap, tmp_2d, f0_ap,
                                                    op=ADD)
                            pool_px[0] = ost[:, k: k + 1, l: l + 1]
                    nc.sync.dma_start(out_d[:, 2 * y0: 2 * y0 + 2 * rr, :],
                                      ost[:])

                # Tail absorbers (see baseline notes): engine-matched writes
                # to the final ost slots carry the WAR waits on their output
                # DMAs, then a DVE read of the last Pool-written slot pulls
                # Pool's clock into DVE's.
                last_chain_t = None
                n_tail = len(ost_tiles) - 6
                for ti in range(max(0, n_tail), len(ost_tiles)):
                    t = ost_tiles[ti]
                    if ti % 2 == 1:
                        nc.gpsimd.memset(t[:, 0:1, 0:1], 0.0)
                        last_chain_t = t
                    else:
                        nc.vector.memset(t[:, 0:1, 0:1], 0.0)
                if last_chain_t is not None:
                    scratch = tmpp.tile([128, 1, 1], bf, tag="scratch",
                                        name="scratch")
                    nc.vector.tensor_copy(scratch[:],
                                          last_chain_t[:, 0:1, 0:1])
                scratch = tmpp.tile([128, 1, B0], bf, tag="scratch_row",
                                    name="scratch_row")
                nc.vector.tensor_copy(scratch[:], f0p[:, 0:1, :])

        body(0, xt)
        for rep in range(1, n_repeat):
            xt = pers.tile([128, B0, B0], bf, tag="pers",
                           name=f"xt_s{rep}")
            dma_input(xt)
            body(rep, xt)

    _legalize_waits(nc, mybir)
    nc.finalize()
    return nc


def _legalize_waits(nc, mybir):
    """Drop semaphore waits provably implied by other synchronization.

    Compute-engine and DMA ISA structs fit only one sync wait, but Tile's
    sem assignment is per-proc minimal, not transitively minimal. We replay
    the schedule with a vector-clock and drop implied waits. DMACopy
    dispatch is asynchronous (DGE evaluates its waits, not the issuing
    engine), so DMAs contribute nothing to engine knowledge and get no
    dispatch-order credit.
    """
    QDEPTH_PE, QDEPTH_OTHER = 64, 8

    def merge(dst, src_):
        for k, v in src_.items():
            if dst.get(k, -1) < v:
                dst[k] = v

    def implies(k, s, v):
        return k.get(s, -1) >= v

    cum = {}          # sem -> cumulative increments so far (schedule order)
    snap = {}         # sem -> list of (post_cum, completion-knowledge dict)
    kw = {}           # engine -> knowledge from dispatch-blocking waits
    kc = {}           # engine -> knowledge from >=Q-back completions
    ring = {}         # engine -> list of (own-increments dict)

    def snap_lookup(s, v):
        lst = snap.get(s)
        if not lst:
            return {}
        lo, hi = 0, len(lst)
        while lo < hi:
            mid = (lo + hi) // 2
            if lst[mid][0] >= v:
                hi = mid
            else:
                lo = mid + 1
        return lst[lo][1] if lo < len(lst) else {}

    for b in nc.m.functions[0].blocks:
        for inst in b.instructions:
            si = inst.sync_info
            eng = str(inst.engine)
            opcode = type(inst).__name__
            is_dma = ("DMACopy" in opcode or "TriggeredCopy" in opcode
                      or "Collective" in opcode)
            waits = list(si.on_wait or []) if si is not None else []
            updates = list(si.on_update or []) if si is not None else []

            is_pool = "Pool" in eng
            if is_dma:
                kdisp = {}
            else:
                if not is_pool:
                    q = QDEPTH_PE if "PE" in eng else QDEPTH_OTHER
                    r = ring.setdefault(eng, [])
                    if len(r) >= q:
                        merge(kc.setdefault(eng, {}), r.pop(0))
                kdisp = dict(kw.get(eng, {}))
                merge(kdisp, kc.get(eng, {}))

            wknow = []
            for w in waits:
                g = dict(snap_lookup(w.ant_name, w.wait_value))
                if g.get(w.ant_name, -1) < w.wait_value:
                    g[w.ant_name] = w.wait_value
                wknow.append(g)

            kept = list(range(len(waits)))
            if len(waits) > 1:
                changed = True
                while changed and len(kept) > 1:
                    changed = False
                    for idx in list(kept):
                        w = waits[idx]
                        if is_pool and "Pool" in w.ant_name:
                            k_union = {}
                            for j in kept:
                                if j != idx:
                                    merge(k_union, wknow[j])
                        else:
                            k_union = dict(kdisp)
                            for j in kept:
                                if j != idx:
                                    merge(k_union, wknow[j])
                        if implies(k_union, w.ant_name, w.wait_value):
                            kept.remove(idx)
                            changed = True
                            break
                if len(kept) < len(waits):
                    inst.sync_info = mybir.SyncInfo(
                        on_wait=[waits[i] for i in kept], on_update=updates)

            k_wait = dict(kdisp)
            for g in wknow:
                merge(k_wait, g)

            own_incs = {}
            for u in updates:
                s = u.ant_name
                cum[s] = cum.get(s, 0) + u.update_value
                own_incs[s] = cum[s]

            if own_incs:
                comp = dict(k_wait)
                merge(comp, own_incs)
                for s, v in own_incs.items():
                    snap.setdefault(s, []).append((v, comp))

            if not is_dma:
                merge(kw.setdefault(eng, {}), k_wait)
                ring.setdefault(eng, []).append(own_incs)


def get_program():
    global _PROGRAM
    if _PROGRAM is None:
        _PROGRAM = _build_program()
    return _PROGRAM


def fold_weights(inputs, fi, fj):
    """Host-side weight transform -> dict of bf16 arrays in kernel layout.
    fi/fj: flip the kernel along H/W; stride-2 convs get their flipped 3x3
    placed at offset (fi,fj) of a 4x4 window (1x1 ds: 2x2 window) to fix
    the stride-2 sampling phase under flip."""
    out = {}
    for n in W3_NAMES + W16_NAMES:
        w = np.asarray(inputs[n], np.float32) * W_SCALE[n]   # [O,I,3,3]
        if fi:
            w = w[:, :, ::-1, :]
        if fj:
            w = w[:, :, :, ::-1]
        if n in W16_NAMES:
            w4 = np.zeros((128, 128, 4, 4), np.float32)
            w4[:, :, fi: fi + 3, fj: fj + 3] = w
            out[n] = np.ascontiguousarray(
                w4.transpose(1, 2, 3, 0).reshape(128, 16, 128)).astype(BF16)
        else:
            out[n] = np.ascontiguousarray(
                w.transpose(1, 2, 3, 0).reshape(128, 9, 128)).astype(BF16)
    for n in ('e1b0ds', 'e2b0ds'):
        w = np.asarray(inputs[n], np.float32)[:, :, 0, 0] * W_SCALE[n]
        w2 = np.zeros((128, 128, 2, 2), np.float32)
        w2[:, :, fi, fj] = w
        out[n] = np.ascontiguousarray(
            w2.transpose(1, 2, 3, 0).reshape(128, 4, 128)).astype(BF16)
    for n in ('dec0w', 'dec1w'):
        w = np.asarray(inputs[n], np.float32) * W_SCALE[n]   # [I,O,2,2]
        if fi:
            w = w[:, :, ::-1, :]
        if fj:
            w = w[:, :, :, ::-1]
        out[n] = np.ascontiguousarray(
            w.transpose(0, 2, 3, 1).reshape(128, 4, 128)).astype(BF16)
    return out


def make_in_maps(inputs):
    x = np.asarray(inputs['x'], np.float32)
    wpacks = {}
    for fi in range(2):
        for fj in range(2):
            folded = fold_weights(inputs, fi, fj)
            wp = np.concatenate(
                [folded[n].reshape(128, -1) for n, _, _ in WPACK_OFFS],
                axis=1)
            assert wp.shape == (128, WPACK_LEN)
            wpacks[(fi, fj)] = wp
    Pimg = np.pad(x, ((0, 0), (0, 0), (1, 1), (1, 1)))
    in_maps = []
    for b in range(2):
        for i in range(2):
            for j in range(2):
                rs, cs = RS2[i], RS2[j]
                xt = Pimg[b, :, rs: rs + B0, cs: cs + B0]
                if i:
                    xt = xt[:, ::-1, :]
                if j:
                    xt = xt[:, :, ::-1]
                xt = np.ascontiguousarray(xt).astype(BF16)
                in_maps.append({'xt': xt, 'wpack': wpacks[(i, j)]})
    return in_maps


def assemble(outs):
    """outs: list of 8 dicts with 'out' [128,128,128] -> [2,128,256,256]."""
    res = np.zeros((2, 128, 256, 256), np.float32)
    idx = 0
    for b in range(2):
        for i in range(2):
            for j in range(2):
                o = np.asarray(outs[idx]['out'], np.float32)
                if i:
                    o = o[:, ::-1, :]
                if j:
                    o = o[:, :, ::-1]
                res[b, :, OWN[i]: OWN[i] + 128, OWN[j]: OWN[j] + 128] = o
                idx += 1
    return res


def run_spmd(inputs, **kwargs):
    from concourse.bass_utils import run_bass_kernel_spmd
    nc = get_program()
    in_maps = make_in_maps(inputs)
    res = run_bass_kernel_spmd(nc, in_maps, core_ids=list(range(8)), **kwargs)
    return res


def kernel(**inputs):
    res = run_spmd(inputs)
    return assemble(res.results)


def bench_exec(inputs, iters=20, warmup=3):
    """Time on-device execution by pipelining async dispatches.

    Replicates bass2jax.run_bass_via_pjrt's shard_map execution, pre-places
    inputs on the 8 devices, and chains donation so repeated executions
    queue back-to-back on the devices. Returns (ns_per_iter, outputs)."""
    import time
    import jax
    from jax.sharding import Mesh, PartitionSpec, NamedSharding
    from jax.experimental.shard_map import shard_map
    import concourse.mybir as mybir
    from concourse.bass2jax import (
        _bass_exec_p, install_neuronx_cc_hook, partition_id_tensor)

    install_neuronx_cc_hook()
    nc = get_program()
    in_maps = make_in_maps(inputs)
    n_cores = len(in_maps)
    partition_name = (nc.partition_id_tensor.name
                      if nc.partition_id_tensor else None)

    in_names, out_names, out_avals, zero_outs = [], [], [], []
    for alloc in nc.m.functions[0].allocations:
        if not isinstance(alloc, mybir.MemoryLocationSet):
            continue
        name = alloc.memorylocations[0].name
        if alloc.kind == "ExternalInput":
            if name != partition_name:
                in_names.append(name)
        elif alloc.kind == "ExternalOutput":
            out_names.append(name)
            shape = tuple(alloc.tensor_shape)
            dtype = mybir.dt.np(alloc.dtype)
            out_avals.append(jax.core.ShapedArray(shape, dtype))
            zero_outs.append(np.zeros(shape, dtype))
    n_params = len(in_names)
    n_outs = len(out_avals)
    in_names_all = in_names + out_names
    if partition_name is not None:
        in_names_all = in_names_all + [partition_name]

    def _body(*args):
        operands = list(args)
        if partition_name is not None:
            operands.append(partition_id_tensor())
        outs = _bass_exec_p.bind(
            *operands,
            out_avals=tuple(out_avals),
            in_names=tuple(in_names_all),
            out_names=tuple(out_names),
            lowering_input_output_aliases=(),
            sim_require_finite=True,
            sim_require_nnan=True,
            nc=nc,
        )
        return tuple(outs)

    devices = jax.devices()[:n_cores]
    mesh = Mesh(np.asarray(devices), ("core",))
    spec = PartitionSpec("core")
    donate = tuple(range(n_params, n_params + n_outs))
    f = jax.jit(
        shard_map(_body, mesh=mesh, in_specs=(spec,) * (n_params + n_outs),
                  out_specs=(spec,) * n_outs, check_rep=False),
        donate_argnums=donate, keep_unused=True)

    sharding = NamedSharding(mesh, spec)
    dev_ins = [
        jax.device_put(
            np.concatenate([np.asarray(m[name]) for m in in_maps], axis=0),
            sharding)
        for name in in_names]
    outs = tuple(
        jax.device_put(np.concatenate([z] * n_cores, axis=0), sharding)
        for z in zero_outs)

    for _ in range(warmup):
        outs = f(*dev_ins, *outs)
    jax.block_until_ready(outs)

    def window(n):
        nonlocal outs
        t0 = time.perf_counter()
        for _ in range(n):
            outs = f(*dev_ins, *outs)
        jax.block_until_ready(outs)
        return time.perf_counter() - t0

    if iters >= 60:
        n1 = iters // 4
        t1 = min(window(n1), window(n1))
        t2 = min(window(iters), window(iters))
        ns = (t2 - t1) / (iters - n1) * 1e9
    else:
        ns = window(iters) / iters * 1e9
    return ns, outs


# revision 28
# speedup vs baseline: 2.8481x; 2.1580x over previous
"""Trainium2 Bass kernel for nn_CascadeDEDBackbone (ResNet-style encoder/decoder,
[2,128,256,256] f32, all convs 128->128ch).

Strategy (self-contained, hardcoded):
  - 8 cores = batch(2) x H-half(2) x W-half(2). Host flip-normalizes every
    tile so its owned 128x128 anchors at local (0,0) (weights flipped per
    core to compensate); each conv computes only the region later stages
    consume (validity shrinks 1px/conv toward the high side).
  - Stride-2 convs are not flip-equivariant (sampling phase), so they use a
    4x4 tap window (16 matmuls) with each core's flipped 3x3 weights placed
    at offset (i,j) by the host; 1x1 downsamples use a 2x2 window. This
    keeps one SPMD program for all 8 cores.
  - Input margin is 7px (vs 24 fully-redundant): two halo exchanges refill
    margins mid-net via three CONCURRENT 2-rank AllReduces (H-pair rows,
    W-pair cols, diagonal-pair corner; +local subtract, exact in f32):
    exchange A after e1b0 (margin 2 at 128-res), exchange B on f1
    (margin 7). Producers emit boundary chunks first and consumers emit
    all interior chunks before any margin strip (the PE stream executes
    in emission order), so the collectives hide under interior compute.
    Exchange DMAs ride gpsimd/SWDGE lanes to dodge the contended HWDGE
    ring-reuse waits (one-wait ISA limit).
  - On-core: a KxK conv = K^2 accumulated 128x128 matmuls over shifted APs
    (channels = partitions). PSUM accumulates f32; VectorE evacuates with
    fused relu/scale/residual-add; activations stay resident in SBUF.
  - Decoder evacuation alternates per chunk between DVE and an ACT->Pool
    chain so the deconv stages stay PE-bound; output is bf16 128x128/core.
"""

import os
import sys

import numpy as np
import ml_dtypes

for _p in ("/opt/trn_rl_repo", "/opt/trn_rl_repo/concourse"):
    if os.path.isdir(_p) and _p not in sys.path:
        sys.path.insert(0, _p)

BF16 = ml_dtypes.bfloat16
BN_S = float(1.0 / np.sqrt(1.0 + 1e-3))

# region geometry: owned 128, host margin 7 (incl 1-px ring -> 136 input)
M0 = 7
B0 = 136
REG_L0 = [134, 133, 132, 131]   # e0b0c1, e0b0c2, e0b1c1, e0b1c2(f0)
REG_L1 = [65, 64, 65, 64]       # e1b0c1, e1b0c2, e1b1c1, e1b1c2(f1)
REG_L2 = [35, 34, 33, 32]       # e2b0c1, e2b0c2, e2b1c1, e2b1c2(e2o)
MA = 2                          # exchange A margin (bo1, 128-res)
MB = 7                          # exchange B margin (f1p, 128-res)
DEC0_IN = 32
DEC1_IN = 64
OWNED = 128
RS2 = [0, 122]                  # host slice start per tile index
OWN = [0, 128]

OFF9 = [(dy, dx) for dy in range(3) for dx in range(3)]
OFF16 = [(dy, dx) for dy in range(4) for dx in range(4)]
OFF4 = [(k, l) for k in range(2) for l in range(2)]

GRP_H = [[0, 2], [1, 3], [4, 6], [5, 7]]
GRP_W = [[0, 1], [2, 3], [4, 5], [6, 7]]
GRP_D = [[0, 3], [1, 2], [4, 7], [5, 6]]

W3_NAMES = ['e0b0c1', 'e0b0c2', 'e0b1c1', 'e0b1c2',
            'e1b0c2', 'e1b1c1', 'e1b1c2',
            'e2b0c2', 'e2b1c1', 'e2b1c2']
W16_NAMES = ['e1b0c1', 'e2b0c1']
# BN-scale folding: weights consuming pre-scaled f0'/f1' buffers get their
# BN fold cancelled.
W_SCALE = {'e0b0c1': BN_S, 'e0b0c2': BN_S, 'e0b1c1': BN_S, 'e0b1c2': BN_S,
           'e1b0c1': 1.0, 'e1b0c2': BN_S, 'e1b0ds': 1.0,
           'e1b1c1': BN_S, 'e1b1c2': BN_S,
           'e2b0c1': 1.0, 'e2b0c2': BN_S, 'e2b0ds': 1.0,
           'e2b1c1': BN_S, 'e2b1c2': BN_S,
           'dec0w': BN_S * BN_S, 'dec1w': BN_S * BN_S}

_W_ORDER = ['e0b0c1', 'e0b0c2', 'e0b1c1', 'e0b1c2',
            'e1b0c1', 'e1b0c2', 'e1b0ds', 'e1b1c1', 'e1b1c2',
            'e2b0c1', 'e2b0c2', 'e2b0ds', 'e2b1c1', 'e2b1c2',
            'dec0w', 'dec1w']
_W_LENS = {**{n: 9 * 128 for n in W3_NAMES},
           **{n: 16 * 128 for n in W16_NAMES},
           'e1b0ds': 4 * 128, 'e2b0ds': 4 * 128,
           'dec0w': 4 * 128, 'dec1w': 4 * 128}
WPACK_OFFS = []
_off = 0
for _n in _W_ORDER:
    WPACK_OFFS.append((_n, _off, _W_LENS[_n]))
    _off += _W_LENS[_n]
WPACK_LEN = _off

_PROGRAM = None  # cached bass.Bass


def _build_program(n_repeat=None):
    import concourse.bass as bass
    import concourse.mybir as mybir
    import concourse.tile as tile
    from contextlib import ExitStack

    if n_repeat is None:
        n_repeat = int(os.environ.get("K_FULL_REPEAT", "1"))
    use_cc = os.environ.get("K_NO_CC", "0") != "1"

    bf = mybir.dt.bfloat16
    f32 = mybir.dt.float32
    ADD = mybir.AluOpType.add
    SUB = mybir.AluOpType.subtract
    MAX = mybir.AluOpType.max
    MULT = mybir.AluOpType.mult

    nc = bass.Bass()

    xt_d = nc.dram_tensor("xt", [128, B0, B0], bf, kind="ExternalInput")
    wpack_d = nc.dram_tensor("wpack", [128, WPACK_LEN], bf, kind="ExternalInput")
    out_d = nc.dram_tensor("out", [128, OWNED, OWNED], bf,
                           kind="ExternalOutput")

    with tile.TileContext(nc) as tc, ExitStack() as ctx:
        wp = ctx.enter_context(tc.tile_pool(name="wpool", bufs=1))
        wslab = wp.tile([128, WPACK_LEN], bf, tag="wpack", name="wslab")
        w3 = {}
        for n, off, ln in WPACK_OFFS:
            view = wslab[:, off: off + ln]
            w3[n] = view.rearrange("c (n m) -> c n m", n=ln // 128)

        def dma_w(n):
            _, off, ln = next(t for t in WPACK_OFFS if t[0] == n)
            nc.sync.dma_start(wslab[:, off: off + ln],
                              wpack_d[:, off: off + ln])

        pers = ctx.enter_context(tc.tile_pool(name="pers", bufs=1))

        psp = ctx.enter_context(
            tc.tile_pool(name="psp", bufs=4, space=bass.MemorySpace.PSUM))
        tmpp = ctx.enter_context(tc.tile_pool(name="tmpp", bufs=8))
        dramp = ctx.enter_context(
            tc.tile_pool(name="dramp", bufs=1, space="DRAM"))
        xch = ctx.enter_context(tc.tile_pool(name="xch", bufs=1))

        # DMA order: first conv's weights, then the input in row bands (so
        # the first conv chunks start as soon as their rows land), then the
        # remaining weights. After each band, a tiny DVE read (absorber)
        # publishes the band's DMA completion into DVE's vector clock.
        _, _w0off, _ = next(t for t in WPACK_OFFS if t[0] == 'e0b0c1')
        nc.sync.dma_start(wslab[:, _w0off: _w0off + 128],
                          wpack_d[:, _w0off: _w0off + 128])
        nc.sync.dma_start(wslab[:, _w0off + 128: _w0off + 9 * 128],
                          wpack_d[:, _w0off + 128: _w0off + 9 * 128])

        def dma_input(xt):
            bands = [0, 3, 6] + list(range(26, B0, 20)) + [B0]
            for a, b_hi in zip(bands[:-1], bands[1:]):
                nc.sync.dma_start(xt[:, a:b_hi, :], xt_d[:, a:b_hi, :])
                scratch = tmpp.tile([128, 1, 1], bf, tag="scratch",
                                    name="scratch")
                nc.vector.tensor_copy(scratch[:], xt[:, b_hi - 1:b_hi, 0:1])

        xt = pers.tile([128, B0, B0], bf, tag="pers", name="xt_s")
        dma_input(xt)
        _w_emit = ['dec1w', 'dec0w', 'e2b0ds', 'e2b1c2', 'e2b1c1', 'e2b0c2',
                   'e2b0c1', 'e1b0ds', 'e1b1c2', 'e1b1c1', 'e1b0c2', 'e1b0c1',
                   'e0b1c2', 'e0b1c1', 'e0b0c2']
        assert set(_w_emit) == {n for n, _, _ in WPACK_OFFS} - {'e0b0c1'}
        for n in _w_emit:
            dma_w(n)
        del dma_w

        def ring_zero(t, H):
            # only the low-side ring (image-edge zero padding) is read
            nc.vector.memset(t[:, 0, :], 0.0)
            nc.vector.memset(t[:, 1:, 0], 0.0)

        def conv3x3(src, dst, w, stride, rchunk, y_off=0, hr=None, x0=0,
                    cw=None, reg=None, scale=1.0, resid=None, extra_mm=None,
                    taps=OFF9):
            """Compute out rows y_off..y_off+hr-1, cols x0..x0+cw-1
            (0-based within interior; dst row = 1+y). reg: full region (for
            defaults). resid: callable (y0, rr, x0, cw) -> identity AP."""
            hr = reg - y_off if hr is None else hr
            cw = reg - x0 if cw is None else cw
            if resid is not None:
                ab = tmpp.tile([128, 1, cw], bf, tag="absorb", name="absorb")
                nc.vector.tensor_copy(ab[:], resid(y_off + hr - 1, 1, x0, cw))
            n_ext = 0 if extra_mm is None else 4
            last = len(taps) + n_ext - 1
            for y0 in range(y_off, y_off + hr, rchunk):
                rr = min(rchunk, y_off + hr - y0)
                ps = psp.tile([128, rr, cw], f32, tag="ps", name="ps")
                for o, (dy, dx) in enumerate(taps):
                    r_lo = y0 * stride + dy
                    c_lo = x0 * stride + dx
                    rhs = src[:, r_lo: r_lo + (rr - 1) * stride + 1: stride,
                              c_lo: c_lo + (cw - 1) * stride + 1: stride]
                    nc.tensor.matmul(ps[:], w[:, o, :], rhs,
                                     start=(o == 0), stop=(o == last))
                if extra_mm is not None:
                    w4, src2 = extra_mm
                    for q, (dy, dx) in enumerate(OFF4):
                        rhs = src2[:, 1 + 2 * y0 + dy:
                                   1 + 2 * y0 + dy + (rr - 1) * 2 + 1: 2,
                                   1 + 2 * x0 + dx:
                                   1 + 2 * x0 + dx + (cw - 1) * 2 + 1: 2]
                        nc.tensor.matmul(ps[:], w4[:, q, :], rhs,
                                         start=False,
                                         stop=(len(taps) + q == last))
                dst_ap = dst[:, 1 + y0: 1 + y0 + rr, 1 + x0: 1 + x0 + cw]
                if resid is None:
                    nc.vector.tensor_scalar(dst_ap, ps[:], 0.0, scale, MAX,
                                            MULT)
                else:
                    tmp = tmpp.tile([128, rr, cw], bf, tag="tmpr", name="tmpr")
                    nc.vector.tensor_tensor(tmp[:], ps[:],
                                            resid(y0, rr, x0, cw), op=ADD)
                    nc.vector.tensor_scalar(dst_ap, tmp[:], 0.0, scale, MAX,
                                            MULT)

        def interior(buf):
            return lambda y0, rr, x0, cw: buf[:, 1 + y0: 1 + y0 + rr,
                                              1 + x0: 1 + x0 + cw]

        def _ar_chain(ox, shp, grp, sfx, tag):
            ci = dramp.tile(shp, f32, tag=f"ci{sfx}{tag}")
            co = dramp.tile(shp, f32, tag=f"co{sfx}{tag}")
            nc.gpsimd.dma_start(ci[:], ox[:])
            if use_cc:
                nc.gpsimd.collective_compute(
                    "AllReduce", ADD, replica_groups=grp,
                    ins=[ci.opt()], outs=[co.opt()])
            else:
                nc.sync.dma_start(co[:], ci[:])
            return co

        def exchange_hd(buf, n, m, tag):
            """Early phase: launch the H-row and diagonal-corner AllReduces.
            Their payloads need only the producer's BOUNDARY chunk (emitted
            first), so issuing here - before the interior conv evacs enter
            the DVE stream - gives the collectives the whole interior
            compute as flight time. SPMD-safe via AllReduce(add) of f32 +
            local subtract (exact)."""
            oxh = xch.tile([128, m, n], f32, tag=f"oxh{tag}", name="oxh")
            for r in range(m):
                nc.vector.tensor_copy(oxh[:, r, :], buf[:, n - r, 1:1 + n])
            oxd = xch.tile([128, m, m], f32, tag=f"oxd{tag}", name="oxd")
            for r in range(m):
                nc.vector.tensor_copy(oxd[:, r, :],
                                      buf[:, n - r, n - m + 1: n + 1])
            coh = _ar_chain(oxh, [128, m, n], GRP_H, "h", tag)
            cod = _ar_chain(oxd, [128, m, m], GRP_D, "d", tag)
            # apply H and corner margins HERE, on the Pool engine: Pool is
            # idle during the exchange, so the (long-latency) subtracts do
            # not head-of-line-block the DVE stream's interior evacs, and
            # the H margins are available as soon as the AR lands.
            inh = xch.tile([128, m, n], f32, tag=f"inh{tag}", name="inh")
            nc.gpsimd.dma_start(inh[:], coh[:])
            nc.gpsimd.tensor_tensor(buf[:, n + 1: n + 1 + m, 1: 1 + n],
                                    inh[:], oxh[:], op=SUB)
            ind = xch.tile([128, m, m], f32, tag=f"ind{tag}", name="ind")
            nc.gpsimd.dma_start(ind[:], cod[:])
            for c in range(m):
                nc.gpsimd.tensor_tensor(
                    buf[:, n + 1: n + 1 + m, n + 1 + c],
                    ind[:, :, m - 1 - c], oxd[:, :, m - 1 - c], op=SUB)

        def exchange_w(buf, n, m, tag):
            """Late phase: W-col AllReduce (needs every row chunk of buf),
            then apply the W margins. W outbox stored TRANSPOSED [m, n]
            so the bounce DMAs burst well (28B inner runs DMA ~5x slower).
            Writes rows 1..n only - disjoint from the corner applied in
            exchange_hd."""
            oxw = xch.tile([128, m, n], f32, tag=f"oxw{tag}", name="oxw")
            for c in range(m):
                nc.vector.tensor_copy(oxw[:, c, :], buf[:, 1: 1 + n, n - c])
            cow = _ar_chain(oxw, [128, m, n], GRP_W, "w", tag)
            inw = xch.tile([128, m, n], f32, tag=f"inw{tag}", name="inw")
            nc.gpsimd.dma_start(inw[:], cow[:])
            # apply on Pool (like H/D): fires the moment the AR lands
            # instead of queueing behind the early-strip evacs on DVE
            for c in range(m):
                nc.gpsimd.tensor_tensor(buf[:, 1: 1 + n, n + 1 + c],
                                        inw[:, c, :], oxw[:, c, :], op=SUB)

        def body(rep, xt):
            rtag = f"r{rep}" if rep else ""
            # ------------- level 0 (256-res, regions 134..131) -----------
            with tc.tile_pool(name=f"work{rtag}", bufs=2) as workp:
                c1 = workp.tile([128, B0, B0], bf, tag="work", name="b0c1")
                ring_zero(c1, B0 - 2)
                for _rep in range(int(os.environ.get("K_CAL_REPEAT", "1"))):
                    conv3x3(xt, c1, w3['e0b0c1'], 1, 3, reg=REG_L0[0])
                b0 = workp.tile([128, B0, B0], bf, tag="work", name="b0out")
                ring_zero(b0, B0 - 2)
                conv3x3(c1, b0, w3['e0b0c2'], 1, 3, reg=REG_L0[1],
                        resid=interior(xt))
                c1 = workp.tile([128, B0, B0], bf, tag="work", name="b1c1")
                ring_zero(c1, B0 - 2)
                conv3x3(b0, c1, w3['e0b1c1'], 1, 3, reg=REG_L0[2])
                f0p = pers.tile([128, B0, B0], bf, tag="pers",
                                name="f0p")
                ring_zero(f0p, B0 - 2)
                conv3x3(c1, f0p, w3['e0b1c2'], 1, 3, reg=REG_L0[3],
                        scale=BN_S, resid=interior(b0))

            # ------------- levels 1+2 / decoder pools -------------
            with tc.tile_pool(name=f"acts2{rtag}", bufs=1) as acts2, \
                 tc.tile_pool(name=f"lv{rtag}", bufs=1) as lv, \
                 tc.tile_pool(name=f"ostp{rtag}", bufs=6) as ostp, \
                 tc.tile_pool(name=f"tmpa{rtag}", bufs=4) as tmpa, \
                 tc.tile_pool(name=f"dpsp{rtag}", bufs=4,
                              space=bass.MemorySpace.PSUM) as dpsp:

                # ---- level 1 block 0 (stride-2, 16-tap window) ----
                c1 = lv.tile([128, 67, 67], bf, tag="c1", name="c1")
                ring_zero(c1, 65)
                conv3x3(f0p, c1, w3['e1b0c1'], 2, 6, reg=REG_L1[0],
                        taps=OFF16)
                bo1 = lv.tile([128, 67, 67], bf, tag="bo1", name="bo1")
                ring_zero(bo1, 65)
                # boundary rows first so exchange A starts early
                conv3x3(c1, bo1, w3['e1b0c2'], 1, 6, y_off=60, hr=4,
                        reg=REG_L1[1], extra_mm=(w3['e1b0ds'], f0p))
                exchange_hd(bo1, 64, MA, f"A{rtag}")
                conv3x3(c1, bo1, w3['e1b0c2'], 1, 6, y_off=0, hr=60,
                        reg=REG_L1[1], extra_mm=(w3['e1b0ds'], f0p))
                exchange_w(bo1, 64, MA, f"A{rtag}")

                # ---- level 1 block 1 (interior + margin strips) ----
                c1b = lv.tile([128, 67, 67], bf, tag="b1c1", name="b1c1")
                ring_zero(c1b, 65)
                conv3x3(bo1, c1b, w3['e1b1c1'], 1, 6, y_off=0, hr=63,
                        x0=0, cw=63, reg=REG_L1[2])
                conv3x3(bo1, c1b, w3['e1b1c1'], 1, 63, y_off=0, hr=63,
                        x0=63, cw=2, reg=REG_L1[2])
                conv3x3(bo1, c1b, w3['e1b1c1'], 1, 6, y_off=63, hr=2,
                        x0=0, cw=65, reg=REG_L1[2])
                f1p = acts2.tile([128, 72, 72], bf, tag="f1p", name="f1p")
                ring_zero(f1p, 70)
                conv3x3(c1b, f1p, w3['e1b1c2'], 1, 6, y_off=54, hr=10,
                        reg=REG_L1[3], scale=BN_S, resid=interior(bo1))
                exchange_hd(f1p, 64, MB, f"B{rtag}")
                conv3x3(c1b, f1p, w3['e1b1c2'], 1, 6, y_off=0, hr=54,
                        reg=REG_L1[3], scale=BN_S, resid=interior(bo1))
                exchange_w(f1p, 64, MB, f"B{rtag}")

                # ---- level 2 ----
                # Emit ALL interiors first, then all margin strips: the PE
                # stream runs in emission order, so strips (which wait on
                # exchange B) must sit behind every interior chunk to avoid
                # head-of-line blocking while the collectives fly.
                c2 = lv.tile([128, 37, 37], bf, tag="c1", name="c2")
                ring_zero(c2, 35)
                bo2 = lv.tile([128, 36, 36], bf, tag="bo1", name="bo2")
                ring_zero(bo2, 34)
                c2b = lv.tile([128, 35, 35], bf, tag="b1c1", name="b2c1")
                ring_zero(c2b, 33)
                e2o = acts2.tile([128, 34, 34], bf, tag="e2o", name="e2o")
                ring_zero(e2o, 32)
                conv3x3(f1p, c2, w3['e2b0c1'], 2, 13, y_off=0, hr=31,
                        x0=0, cw=31, reg=REG_L2[0], taps=OFF16)
                conv3x3(c2, bo2, w3['e2b0c2'], 1, 13, y_off=0, hr=30,
                        x0=0, cw=30, reg=REG_L2[1],
                        extra_mm=(w3['e2b0ds'], f1p))
                conv3x3(bo2, c2b, w3['e2b1c1'], 1, 13, y_off=0, hr=29,
                        x0=0, cw=29, reg=REG_L2[2])
                conv3x3(c2b, e2o, w3['e2b1c2'], 1, 13, y_off=0, hr=28,
                        x0=0, cw=28, reg=REG_L2[3], resid=interior(bo2))
                # EARLY strips: bottom-left portions read only the H
                # margin, applied early on Pool by exchange_hd - they fill
                # the W-AR window before the W/corner-dependent strips.
                conv3x3(f1p, c2, w3['e2b0c1'], 2, 13, y_off=31, hr=4,
                        x0=0, cw=31, reg=REG_L2[0], taps=OFF16)
                conv3x3(c2, bo2, w3['e2b0c2'], 1, 13, y_off=30, hr=4,
                        x0=0, cw=30, reg=REG_L2[1],
                        extra_mm=(w3['e2b0ds'], f1p))
                conv3x3(bo2, c2b, w3['e2b1c1'], 1, 13, y_off=29, hr=4,
                        x0=0, cw=29, reg=REG_L2[2])
                conv3x3(c2b, e2o, w3['e2b1c2'], 1, 13, y_off=28, hr=4,
                        x0=0, cw=28, reg=REG_L2[3], resid=interior(bo2))
                # LATE strips (need W/corner margins), cascade order
                conv3x3(f1p, c2, w3['e2b0c1'], 2, 31, y_off=0, hr=31,
                        x0=31, cw=4, reg=REG_L2[0], taps=OFF16)
                conv3x3(f1p, c2, w3['e2b0c1'], 2, 13, y_off=31, hr=4,
                        x0=31, cw=4, reg=REG_L2[0], taps=OFF16)
                conv3x3(c2, bo2, w3['e2b0c2'], 1, 30, y_off=0, hr=30,
                        x0=30, cw=4, reg=REG_L2[1],
                        extra_mm=(w3['e2b0ds'], f1p))
                conv3x3(c2, bo2, w3['e2b0c2'], 1, 13, y_off=30, hr=4,
                        x0=30, cw=4, reg=REG_L2[1],
                        extra_mm=(w3['e2b0ds'], f1p))
                conv3x3(bo2, c2b, w3['e2b1c1'], 1, 29, y_off=0, hr=29,
                        x0=29, cw=4, reg=REG_L2[2])
                conv3x3(bo2, c2b, w3['e2b1c1'], 1, 13, y_off=29, hr=4,
                        x0=29, cw=4, reg=REG_L2[2])
                conv3x3(c2b, e2o, w3['e2b1c2'], 1, 28, y_off=0, hr=28,
                        x0=28, cw=4, reg=REG_L2[3], resid=interior(bo2))
                conv3x3(c2b, e2o, w3['e2b1c2'], 1, 13, y_off=28, hr=4,
                        x0=28, cw=4, reg=REG_L2[3], resid=interior(bo2))

                # ---------------- decoder ----------------
                RELU = mybir.ActivationFunctionType.Relu
                COPY = mybir.ActivationFunctionType.Copy

                pool_px = [None]

                def act_absorb():
                    if pool_px[0] is not None:
                        ab = tmpa.tile([128, 1], bf, tag="actab", name="actab")
                        nc.scalar.activation(ab[:], pool_px[0], COPY)

                # stage 0: y1 = BN_S*relu(deconv(e2o, dec0w*s)) + f1p
                D0, D1 = DEC0_IN, DEC1_IN
                y1 = acts2.tile([128, D1, D1], bf, tag="y1", name="y1")
                use_chain = os.environ.get("K_DEC_CHAIN", "1") == "1"
                for ci, y0 in enumerate(range(0, D0, 9)):
                    chain = (ci % 2 == 1) and use_chain
                    if chain:
                        act_absorb()
                    for p, (k, l) in enumerate(OFF4):
                        rr = min(9, D0 - y0)
                        pool = psp if p % 2 == 0 else dpsp
                        tag = "ps" if p % 2 == 0 else "dps"
                        ps = pool.tile([128, rr, D0], f32, tag=tag, name="ps")
                        nc.tensor.matmul(ps[:], w3['dec0w'][:, p, :],
                                         e2o[:, 1 + y0: 1 + y0 + rr, 1: 1 + D0])
                        y1_ap = y1[:, 2 * y0 + k: 2 * (y0 + rr - 1) + k + 1: 2,
                                   l: 2 * (D0 - 1) + l + 1: 2]
                        f1_ap = f1p[:, 1 + 2 * y0 + k:
                                    1 + 2 * (y0 + rr - 1) + k + 1: 2,
                                    1 + l: 1 + 2 * (D0 - 1) + l + 1: 2]
                        # s^2 folded into dec0w: relu(s^2 psum) = s relu(s psum)
                        if not chain:
                            nc.vector.scalar_tensor_tensor(
                                y1_ap, ps[:], 0.0, f1_ap, op0=MAX, op1=ADD)
                        else:
                            tmp = tmpa.tile([128, rr, D0], bf, tag="t0",
                                            name="t0", bufs=8)
                            nc.scalar.activation(tmp[:], ps[:], RELU)
                            if p == 0:
                                pab = tmpa.tile([128, 1], bf,
                                                tag=f"pab0_{ci}{rtag}",
                                                name="pab", bufs=1)
                                nc.gpsimd.tensor_copy(pab[:], tmp[:, 0, 0:1])
                            nc.gpsimd.tensor_tensor(y1_ap, tmp[:], f1_ap,
                                                    op=ADD)
                            pool_px[0] = y1[:, 2 * y0 + k: 2 * y0 + k + 1,
                                            l: l + 1]

                # stage 1: out = BN_S*relu(deconv(y1, dec1w*s)) + f0p
                ost_tiles = []
                SUB6 = 6
                nch = (D1 + SUB6 - 1) // SUB6

                def ensure_ost(i):
                    while len(ost_tiles) <= min(i, nch - 1):
                        j = len(ost_tiles)
                        rrj = min(SUB6, D1 - j * SUB6)
                        t = ostp.tile([128, 2 * rrj, OWNED], bf, tag="ost",
                                      name="ost")
                        ost_tiles.append(t)
                        if j % 2 == 0:
                            nc.vector.memset(t[:, 0:1, 0:1], 0.0)
                        else:
                            nc.gpsimd.memset(t[:, 0:1, 0:1], 0.0)

                for ci, y0 in enumerate(range(0, D1, SUB6)):
                    ensure_ost(ci + 4)
                    chain = (ci % 2 == 1) and use_chain
                    rr = min(SUB6, D1 - y0)
                    ost = ost_tiles[ci]
                    if chain:
                        act_absorb()
                    for p, (k, l) in enumerate(OFF4):
                        ps = dpsp.tile([128, rr * D1], f32, tag="dps",
                                       name="dps")
                        nc.tensor.matmul(ps[:], w3['dec1w'][:, p, :],
                                         y1[:, y0: y0 + rr, :])
                        f0_ap = f0p[:, 1 + 2 * y0 + k:
                                    1 + 2 * (y0 + rr - 1) + k + 1: 2,
                                    1 + l: 1 + 2 * (D1 - 1) + l + 1: 2]
                        ost_ap = ost[:, k: 2 * (rr - 1) + k + 1: 2,
                                     l: 2 * (D1 - 1) + l + 1: 2]
                        ps_2d = ps[:].rearrange("c (r w) -> c r w", r=rr)
                        # s^2 folded into dec1w
                        if not chain:
                            nc.vector.scalar_tensor_tensor(
                                ost_ap, ps_2d, 0.0, f0_ap, op0=MAX, op1=ADD)
                        else:
                            tmp = tmpa.tile([128, rr * D1], bf, tag="t1",
                                            name="t1", bufs=8)
                            nc.scalar.activation(tmp[:], ps[:], RELU)
                            if p == 0:
                                pab = tmpa.tile([128, 1], bf,
                                                tag=f"pab1_{ci}{rtag}",
                                                name="pab", bufs=1)
                                nc.gpsimd.tensor_copy(pab[:], tmp[:, 0:1])
                            tmp_2d = tmp[:].rearrange("c (r w) -> c r w", r=rr)
                            nc.gpsimd.tensor_tensor(ost_ap, tmp_2d, f0_ap,
                                                    op=ADD)
                            pool_px[0] = ost[:, k: k + 1, l: l + 1]
                    nc.sync.dma_start(out_d[:, 2 * y0: 2 * y0 + 2 * rr, :],
                                      ost[:])

                # Tail absorbers (see baseline notes): engine-matched writes
                # to the final ost slots carry the WAR waits on their output
                # DMAs, then a DVE read of the last Pool-written slot pulls
                # Pool's clock into DVE's.
                last_chain_t = None
                n_tail = len(ost_tiles) - 6
                for ti in range(max(0, n_tail), len(ost_tiles)):
                    t = ost_tiles[ti]
                    if ti % 2 == 1:
                        nc.gpsimd.memset(t[:, 0:1, 0:1], 0.0)
                        last_chain_t = t
                    else:
                        nc.vector.memset(t[:, 0:1, 0:1], 0.0)
                if last_chain_t is not None:
                    scratch = tmpp.tile([128, 1, 1], bf, tag="scratch",
                                        name="scratch")
                    nc.vector.tensor_copy(scratch[:],
                                          last_chain_t[:, 0:1, 0:1])
                scratch = tmpp.tile([128, 1, B0], bf, tag="scratch_row",
                                    name="scratch_row")
                nc.vector.tensor_copy(scratch[:], f0p[:, 0:1, :])

        body(0, xt)
        for rep in range(1, n_repeat):
            xt = pers.tile([128, B0, B0], bf, tag="pers",
                           name=f"xt_s{rep}")
            dma_input(xt)
            body(rep, xt)

    _legalize_waits(nc, mybir)
    nc.finalize()
    return nc


def _legalize_waits(nc, mybir):
    """Drop semaphore waits provably implied by other synchronization.

    Compute-engine and DMA ISA structs fit only one sync wait, but Tile's
    sem assignment is per-proc minimal, not transitively minimal. We replay
    the schedule with a vector-clock and drop implied waits. DMACopy
    dispatch is asynchronous (DGE evaluates its waits, not the issuing
    engine), so DMAs contribute nothing to engine knowledge and get no
    dispatch-order credit.
    """
    QDEPTH_PE, QDEPTH_OTHER = 64, 8

    def merge(dst, src_):
        for k, v in src_.items():
            if dst.get(k, -1) < v:
                dst[k] = v

    def implies(k, s, v):
        return k.get(s, -1) >= v

    cum = {}          # sem -> cumulative increments so far (schedule order)
    snap = {}         # sem -> list of (post_cum, completion-knowledge dict)
    kw = {}           # engine -> knowledge from dispatch-blocking waits
    kc = {}           # engine -> knowledge from >=Q-back completions
    ring = {}         # engine -> list of (own-increments dict)

    def snap_lookup(s, v):
        lst = snap.get(s)
        if not lst:
            return {}
        lo, hi = 0, len(lst)
        while lo < hi:
            mid = (lo + hi) // 2
            if lst[mid][0] >= v:
                hi = mid
            else:
                lo = mid + 1
        return lst[lo][1] if lo < len(lst) else {}

    for b in nc.m.functions[0].blocks:
        for inst in b.instructions:
            si = inst.sync_info
            eng = str(inst.engine)
            opcode = type(inst).__name__
            is_dma = ("DMACopy" in opcode or "TriggeredCopy" in opcode
                      or "Collective" in opcode)
            waits = list(si.on_wait or []) if si is not None else []
            updates = list(si.on_update or []) if si is not None else []

            is_pool = "Pool" in eng
            if is_dma:
                kdisp = {}
            else:
                if not is_pool:
                    q = QDEPTH_PE if "PE" in eng else QDEPTH_OTHER
                    r = ring.setdefault(eng, [])
                    if len(r) >= q:
                        merge(kc.setdefault(eng, {}), r.pop(0))
                kdisp = dict(kw.get(eng, {}))
                merge(kdisp, kc.get(eng, {}))

            wknow = []
            for w in waits:
                g = dict(snap_lookup(w.ant_name, w.wait_value))
                if g.get(w.ant_name, -1) < w.wait_value:
                    g[w.ant_name] = w.wait_value
                wknow.append(g)

            kept = list(range(len(waits)))
            if len(waits) > 1:
                changed = True
                while changed and len(kept) > 1:
                    changed = False
                    for idx in list(kept):
                        w = waits[idx]
                        if is_pool and "Pool" in w.ant_name:
                            k_union = {}
                            for j in kept:
                                if j != idx:
                                    merge(k_union, wknow[j])
                        else:
                            k_union = dict(kdisp)
                            for j in kept:
                                if j != idx:
                                    merge(k_union, wknow[j])
                        if implies(k_union, w.ant_name, w.wait_value):
                            kept.remove(idx)
                            changed = True
                            break
                if len(kept) < len(waits):
                    inst.sync_info = mybir.SyncInfo(
                        on_wait=[waits[i] for i in kept], on_update=updates)

            k_wait = dict(kdisp)
            for g in wknow:
                merge(k_wait, g)

            own_incs = {}
            for u in updates:
                s = u.ant_name
                cum[s] = cum.get(s, 0) + u.update_value
                own_incs[s] = cum[s]

            if own_incs:
                comp = dict(k_wait)
                merge(comp, own_incs)
                for s, v in own_incs.items():
                    snap.setdefault(s, []).append((v, comp))

            if not is_dma:
                merge(kw.setdefault(eng, {}), k_wait)
                ring.setdefault(eng, []).append(own_incs)


def get_program():
    global _PROGRAM
    if _PROGRAM is None:
        _PROGRAM = _build_program()
    return _PROGRAM


def fold_weights(inputs, fi, fj):
    """Host-side weight transform -> dict of bf16 arrays in kernel layout.
    fi/fj: flip the kernel along H/W; stride-2 convs get their flipped 3x3
    placed at offset (fi,fj) of a 4x4 window (1x1 ds: 2x2 window) to fix
    the stride-2 sampling phase under flip."""
    out = {}
    for n in W3_NAMES + W16_NAMES:
        w = np.asarray(inputs[n], np.float32) * W_SCALE[n]   # [O,I,3,3]
        if fi:
            w = w[:, :, ::-1, :]
        if fj:
            w = w[:, :, :, ::-1]
        if n in W16_NAMES:
            w4 = np.zeros((128, 128, 4, 4), np.float32)
            w4[:, :, fi: fi + 3, fj: fj + 3] = w
            out[n] = np.ascontiguousarray(
                w4.transpose(1, 2, 3, 0).reshape(128, 16, 128)).astype(BF16)
        else:
            out[n] = np.ascontiguousarray(
                w.transpose(1, 2, 3, 0).reshape(128, 9, 128)).astype(BF16)
    for n in ('e1b0ds', 'e2b0ds'):
        w = np.asarray(inputs[n], np.float32)[:, :, 0, 0] * W_SCALE[n]
        w2 = np.zeros((128, 128, 2, 2), np.float32)
        w2[:, :, fi, fj] = w
        out[n] = np.ascontiguousarray(
            w2.transpose(1, 2, 3, 0).reshape(128, 4, 128)).astype(BF16)
    for n in ('dec0w', 'dec1w'):
        w = np.asarray(inputs[n], np.float32) * W_SCALE[n]   # [I,O,2,2]
        if fi:
            w = w[:, :, ::-1, :]
        if fj:
            w = w[:, :, :, ::-1]
        out[n] = np.ascontiguousarray(
            w.transpose(0, 2, 3, 1).reshape(128, 4, 128)).astype(BF16)
    return out


def make_in_maps(inputs):
    x = np.asarray(inputs['x'], np.float32)
    wpacks = {}
    for fi in range(2):
        for fj in range(2):
            folded = fold_weights(inputs, fi, fj)
            wp = np.concatenate(
                [folded[n].reshape(128, -1) for n, _, _ in WPACK_OFFS],
                axis=1)
            assert wp.shape == (128, WPACK_LEN)
            wpacks[(fi, fj)] = wp
    Pimg = np.pad(x, ((0, 0), (0, 0), (1, 1), (1, 1)))
    in_maps = []
    for b in range(2):
        for i in range(2):
            for j in range(2):
                rs, cs = RS2[i], RS2[j]
                xt = Pimg[b, :, rs: rs + B0, cs: cs + B0]
                if i:
                    xt = xt[:, ::-1, :]
                if j:
                    xt = xt[:, :, ::-1]
                xt = np.ascontiguousarray(xt).astype(BF16)
                in_maps.append({'xt': xt, 'wpack': wpacks[(i, j)]})
    return in_maps


def assemble(outs):
    """outs: list of 8 dicts with 'out' [128,128,128] -> [2,128,256,256]."""
    res = np.zeros((2, 128, 256, 256), np.float32)
    idx = 0
    for b in range(2):
        for i in range(2):
            for j in range(2):
                o = np.asarray(outs[idx]['out'], np.float32)
                if i:
                    o = o[:, ::-1, :]
                if j:
                    o = o[:, :, ::-1]
                res[b, :, OWN[i]: OWN[i] + 128, OWN[j]: OWN[j] + 128] = o
                idx += 1
    return res


def run_spmd(inputs, **kwargs):
    from concourse.bass_utils import run_bass_kernel_spmd
    nc = get_program()
    in_maps = make_in_maps(inputs)
    res = run_bass_kernel_spmd(nc, in_maps, core_ids=list(range(8)), **kwargs)
    return res


def kernel(**inputs):
    res = run_spmd(inputs)
    return assemble(res.results)


def bench_exec(inputs, iters=20, warmup=3):
    """Time on-device execution by pipelining async dispatches.

    Replicates bass2jax.run_bass_via_pjrt's shard_map execution, pre-places
    inputs on the 8 devices, and chains donation so repeated executions
    queue back-to-back on the devices. Returns (ns_per_iter, outputs)."""
    import time
    import jax
    from jax.sharding import Mesh, PartitionSpec, NamedSharding
    from jax.experimental.shard_map import shard_map
    import concourse.mybir as mybir
    from concourse.bass2jax import (
        _bass_exec_p, install_neuronx_cc_hook, partition_id_tensor)

    install_neuronx_cc_hook()
    nc = get_program()
    in_maps = make_in_maps(inputs)
    n_cores = len(in_maps)
    partition_name = (nc.partition_id_tensor.name
                      if nc.partition_id_tensor else None)

    in_names, out_names, out_avals, zero_outs = [], [], [], []
    for alloc in nc.m.functions[0].allocations:
        if not isinstance(alloc, mybir.MemoryLocationSet):
            continue
        name = alloc.memorylocations[0].name
        if alloc.kind == "ExternalInput":
            if name != partition_name:
                in_names.append(name)
        elif alloc.kind == "ExternalOutput":
            out_names.append(name)
            shape = tuple(alloc.tensor_shape)
            dtype = mybir.dt.np(alloc.dtype)
            out_avals.append(jax.core.ShapedArray(shape, dtype))
            zero_outs.append(np.zeros(shape, dtype))
    n_params = len(in_names)
    n_outs = len(out_avals)
    in_names_all = in_names + out_names
    if partition_name is not None:
        in_names_all = in_names_all + [partition_name]

    def _body(*args):
        operands = list(args)
        if partition_name is not None:
            operands.append(partition_id_tensor())
        outs = _bass_exec_p.bind(
            *operands,
            out_avals=tuple(out_avals),
            in_names=tuple(in_names_all),
            out_names=tuple(out_names),
            lowering_input_output_aliases=(),
            sim_require_finite=True,
            sim_require_nnan=True,
            nc=nc,
        )
        return tuple(outs)

    devices = jax.devices()[:n_cores]
    mesh = Mesh(np.asarray(devices), ("core",))
    spec = PartitionSpec("core")
    donate = tuple(range(n_params, n_params + n_outs))
    f = jax.jit(
        shard_map(_body, mesh=mesh, in_specs=(spec,) * (n_params + n_outs),
                  out_specs=(spec,) * n_outs, check_rep=False),
        donate_argnums=donate, keep_unused=True)

    sharding = NamedSharding(mesh, spec)
    dev_ins = [
        jax.device_put(
            np.concatenate([np.asarray(m[name]) for m in in_maps], axis=0),
            sharding)
        for name in in_names]
    outs = tuple(
        jax.device_put(np.concatenate([z] * n_cores, axis=0), sharding)
        for z in zero_outs)

    for _ in range(warmup):
        outs = f(*dev_ins, *outs)
    jax.block_until_ready(outs)

    def window(n):
        nonlocal outs
        t0 = time.perf_counter()
        for _ in range(n):
            outs = f(*dev_ins, *outs)
        jax.block_until_ready(outs)
        return time.perf_counter() - t0

    if iters >= 60:
        n1 = iters // 4
        t1 = min(window(n1), window(n1))
        t2 = min(window(iters), window(iters))
        ns = (t2 - t1) / (iters - n1) * 1e9
    else:
        ns = window(iters) / iters * 1e9
    return ns, outs
